# revision 15
# baseline (speedup 1.0000x reference)
"""Trainium2 Bass kernel for CausalSelfAttention (B=4, T=2048, C=1024, H=16, D=64).

Sharding: tensor-parallel over attention heads — 2 heads per core, 8 cores,
zero collectives. Each core computes QKV for its 2 heads (full token range),
runs causal attention, and produces a partial output projection
(its heads' columns of W_proj); the host sums the 8 partials and adds b_proj.
The mixed value tensor (an output of the module) is emitted per-core and
reassembled on the host.

Per-core dataflow (token-major QKV -> norm/rope -> PE transpose to dim-major
-> scores^T [k,q] -> exp (no max subtraction needed: qk-norm bounds scores)
-> attn@v with a ones-column to accumulate the softmax denominator ->
normalize -> output projection).
"""

import numpy as np
import ml_dtypes

import concourse.bass as bass
from concourse import bacc, mybir, tile, masks
from concourse.bass_utils import run_bass_kernel_spmd

dt = mybir.dt
AF = mybir.ActivationFunctionType
ALU = mybir.AluOpType

B, T, C, H, D = 4, 2048, 1024, 16, 64
NCORES = 8
HPC = H // NCORES          # heads per core
HD = HPC * D               # 128 head dims per core
ROPE_BASE = 10000.0
KC = C // 128              # contraction chunks for qkv proj


def build_module(Bv=B, Tv=T, debug_taps=False):
    """Build + compile the per-core Bass module. Identical on all cores (SPMD);
    only the input data differs per core."""
    NT = Bv * Tv
    TPB = Tv // 128        # token tiles per batch
    JPB = Tv // 512        # 512-wide q groups per batch
    QKW = 3 * HD           # 384 qkv output dims per core

    nc = bacc.Bacc("TRN2", target_bir_lowering=False, debug=False)
    if debug_taps:
        dbg_qT = nc.dram_tensor("dbg_qT", (128, NT), dt.bfloat16, kind="ExternalOutput").ap()
        dbg_kT = nc.dram_tensor("dbg_kT", (128, NT), dt.bfloat16, kind="ExternalOutput").ap()
        dbg_aT = nc.dram_tensor("dbg_aT", (128, NT), dt.bfloat16, kind="ExternalOutput").ap()
        dbg_es = nc.dram_tensor("dbg_es", (128, 1024), dt.bfloat16, kind="ExternalOutput").ap()
        dbg_psO = nc.dram_tensor("dbg_psO", (65, 512), dt.float32, kind="ExternalOutput").ap()

    xT_h = nc.dram_tensor("xT", (C, NT), dt.bfloat16, kind="ExternalInput").ap()
    wq_h = nc.dram_tensor("wqkvT", (C, QKW), dt.bfloat16, kind="ExternalInput").ap()
    br_h = nc.dram_tensor("brow", (1, QKW), dt.bfloat16, kind="ExternalInput").ap()
    on_h = nc.dram_tensor("ones", (1, 128), dt.bfloat16, kind="ExternalInput").ap()
    v1_h = nc.dram_tensor("v1s", (NT, HD), dt.float32, kind="ExternalInput").ap()
    cs_h = nc.dram_tensor("cs", (128, 4 * TPB * 32), dt.float32, kind="ExternalInput").ap()
    mk_h = nc.dram_tensor("masku", (128, 128), dt.bfloat16, kind="ExternalInput").ap()
    wp_h = nc.dram_tensor("wp", (HD, C), dt.bfloat16, kind="ExternalInput").ap()
    outp_h = nc.dram_tensor("outp", (NT, C), dt.float32, kind="ExternalOutput").ap()
    val_h = nc.dram_tensor("val", (NT, HD), dt.float32, kind="ExternalOutput").ap()

    f32r = dt.float32r

    with tile.TileContext(nc) as tc:
        with (
            tc.tile_pool(name="const", bufs=1) as const,
            tc.tile_pool(name="work", bufs=1) as work,
            tc.tile_pool(name="io", bufs=1) as io,
            tc.tile_pool(name="ps", bufs=1, space="PSUM") as ps,
        ):
            # ---- constants / persistent tensors ----
            wq_sb = const.tile([128, KC * QKW], dt.bfloat16, name="wq_sb")
            for kc in range(KC):
                nc.sync.dma_start(
                    wq_sb[:, kc * QKW:(kc + 1) * QKW],
                    wq_h[kc * 128:(kc + 1) * 128, :],
                )
            br_sb = const.tile([1, QKW], dt.bfloat16, name="br_sb")
            nc.sync.dma_start(br_sb[:], br_h[:])
            on_sb = const.tile([1, 128], dt.bfloat16, name="on_sb")
            nc.sync.dma_start(on_sb[:], on_h[:])
            wp_sb = const.tile([HD, C], dt.bfloat16, name="wp_sb")
            nc.sync.dma_start(wp_sb[:], wp_h[:])
            cs_sb = const.tile([128, 4 * TPB * 32], dt.float32, name="cs_sb")
            nc.sync.dma_start(cs_sb[:], cs_h[:])
            mk_sb = const.tile([128, 128], dt.bfloat16, name="mk_sb")
            nc.sync.dma_start(mk_sb[:], mk_h[:])
            id_sb = const.tile([128, 128], dt.bfloat16, name="id_sb")
            masks.make_identity(nc, id_sb[:])
            eps_sb = const.tile([128, 1], dt.float32, name="eps_sb")
            nc.vector.memset(eps_sb[:], 1e-6)

            qT_sb = const.tile([128, NT], dt.bfloat16, name="qT_sb")
            kT_sb = const.tile([128, NT], dt.bfloat16, name="kT_sb")
            aT_sb = const.tile([128, NT], dt.bfloat16, name="aT_sb")

            U = 2 * TPB  # rope groups (q tiles then k tiles)

            G = 2 * Tv // 64
            state = {}

            def phaseA(b):
                """QKV projection (token-major) for batch b."""
                qksb = work.tile([128, 2 * Tv], dt.float32, tag="qksb", bufs=2, name="qksb")
                vaug = work.tile([128, TPB * 130], dt.bfloat16, tag="vaug", bufs=2, name="vaug")
                nc.vector.memset(
                    vaug[:].rearrange("p (t h y) -> p t h y", h=2, y=65)[:, :, :, 64:65],
                    1.0,
                )
                for m4 in range(TPB // 4):
                    xt = io.tile([128, KC * 512], dt.bfloat16, tag="xt", bufs=2, name="xt")
                    g4 = b * TPB + m4 * 4
                    for kc in range(KC):
                        nc.gpsimd.dma_start(
                            xt[:, kc * 512:(kc + 1) * 512],
                            xT_h[kc * 128:(kc + 1) * 128, g4 * 128:(g4 + 4) * 128],
                        )
                    v1t = io.tile([128, 512], dt.float32, tag="v1t", bufs=2, name="v1t")
                    nc.gpsimd.dma_start(
                        v1t[:].rearrange("p (m d) -> p m d", d=HD),
                        v1_h[g4 * 128:(g4 + 4) * 128, :].rearrange("(m p) d -> p m d", p=128),
                    )
                    vmix4 = io.tile([128, 512], dt.float32, tag="vmix", bufs=2, name="vmix4")
                    for mi in range(4):
                        m = m4 * 4 + mi
                        ps_qkv = ps.tile([128, QKW], dt.float32, tag="mm", bufs=2, name="ps_qkv")
                        for kc in range(KC):
                            nc.tensor.matmul(
                                ps_qkv[:],
                                xt[:, kc * 512 + mi * 128: kc * 512 + (mi + 1) * 128],
                                wq_sb[:, kc * QKW:(kc + 1) * QKW],
                                start=(kc == 0),
                                stop=False,
                            )
                        nc.tensor.matmul(ps_qkv[:], on_sb[:], br_sb[:], start=False, stop=True)
                        nc.scalar.copy(
                            qksb[:].rearrange("p (s t) -> p s t", s=2)[:, :, m * 128:(m + 1) * 128],
                            ps_qkv[:, 0:256].rearrange("p (s u) -> p s u", s=2),
                        )
                        nc.vector.tensor_tensor(
                            vmix4[:, mi * HD:(mi + 1) * HD], ps_qkv[:, 256:384],
                            v1t[:, mi * HD:(mi + 1) * HD], ALU.add,
                        )
                    nc.gpsimd.dma_start(
                        val_h[g4 * 128:(g4 + 4) * 128, :].rearrange("(m p) d -> p m d", p=128),
                        vmix4[:].rearrange("p (m d) -> p m d", d=HD),
                    )
                    nc.vector.tensor_copy(
                        vaug[:, m4 * 520:(m4 + 1) * 520].rearrange("p (m h y) -> p m h y", m=4, y=65)[:, :, :, 0:64],
                        vmix4[:].rearrange("p (m h d) -> p m h d", h=2, d=64),
                    )
                state[("qksb", b)] = qksb
                state[("vaug", b)] = vaug

            def phaseB(b):
                """rms-norm + rope for batch b."""
                qksb = state.pop(("qksb", b))
                sq = work.tile([128, 2 * Tv], dt.float32, tag="qkn", bufs=1, name="sq")
                nc.vector.tensor_tensor(sq[:], qksb[:], qksb[:], ALU.mult)
                ssum = work.tile([128, G], dt.float32, tag="ss", bufs=1, name="ssum")
                nc.vector.tensor_reduce(
                    ssum[:], sq[:].rearrange("p (g d) -> p g d", d=64),
                    mybir.AxisListType.X, ALU.add,
                )
                sfac = work.tile([128, G], dt.float32, tag="sf", bufs=1, name="sfac")
                nc.scalar.activation(sfac[:], ssum[:], AF.Sqrt, scale=1.0 / 64, bias=eps_sb[:])
                rfac = work.tile([128, G], dt.float32, tag="rf", bufs=1, name="rfac")
                nc.vector.reciprocal(rfac[:], sfac[:])
                qkn = work.tile([128, 2 * Tv], dt.float32, tag="qkn", bufs=1, name="qkn")
                nc.vector.tensor_tensor(
                    qkn[:].rearrange("p (g d) -> p g d", d=64),
                    qksb[:].rearrange("p (g d) -> p g d", d=64),
                    rfac[:].broadcast_to([128, G, 64]),
                    ALU.mult,
                )
                rbf = work.tile([128, 2 * Tv], dt.bfloat16, tag="rbf", bufs=1, name="rbf")
                qk4 = qkn[:].rearrange("p (u h d) -> p u h d", h=2, d=64)
                rb4 = rbf[:].rearrange("p (u h d) -> p u h d", h=2, d=64)
                cosv = cs_sb[:, 0:U * 32].rearrange("p (u f) -> p u f", f=32)
                sinv = cs_sb[:, 2 * TPB * 32:2 * TPB * 32 + U * 32].rearrange("p (u f) -> p u f", f=32)
                for h in range(2):
                    x1 = qk4[:, :, h, 0:32]
                    x2 = qk4[:, :, h, 32:64]
                    t1 = work.tile([128, U * 32], dt.float32, tag="tmp", bufs=2, name="t1")
                    t2 = work.tile([128, U * 32], dt.float32, tag="tmp", bufs=2, name="t2")
                    t1v = t1[:].rearrange("p (u f) -> p u f", f=32)
                    t2v = t2[:].rearrange("p (u f) -> p u f", f=32)
                    nc.vector.tensor_tensor(t1v, x1, cosv, ALU.mult)
                    nc.vector.tensor_tensor(t2v, x2, sinv, ALU.mult)
                    nc.vector.tensor_tensor(rb4[:, :, h, 0:32], t1v, t2v, ALU.add)
                    t3 = work.tile([128, U * 32], dt.float32, tag="tmp", bufs=2, name="t3")
                    t4 = work.tile([128, U * 32], dt.float32, tag="tmp", bufs=2, name="t4")
                    t3v = t3[:].rearrange("p (u f) -> p u f", f=32)
                    t4v = t4[:].rearrange("p (u f) -> p u f", f=32)
                    nc.vector.tensor_tensor(t3v, x2, cosv, ALU.mult)
                    nc.vector.tensor_tensor(t4v, x1, sinv, ALU.mult)
                    nc.vector.tensor_tensor(rb4[:, :, h, 32:64], t3v, t4v, ALU.subtract)
                state[("rbf", b)] = rbf

            def phaseC(b):
                """PE transpose to dim-major."""
                rbf = state.pop(("rbf", b))
                for half, dst in ((0, qT_sb), (1, kT_sb)):
                    for p4 in range(TPB // 4):
                        tp = ps.tile([128, 512], dt.bfloat16, tag="mm", bufs=2, name="tp")
                        for t4 in range(4):
                            m = p4 * 4 + t4
                            nc.tensor.transpose(
                                tp[:, t4 * 128:(t4 + 1) * 128],
                                rbf[:, half * Tv + m * 128: half * Tv + (m + 1) * 128],
                                id_sb[:],
                            )
                        nc.vector.tensor_copy(dst[:, b * Tv + p4 * 512: b * Tv + (p4 + 1) * 512], tp[:])

            def phaseD(b):
                """Causal attention for batch b."""
                vaug = state.pop(("vaug", b))
                for j in range(JPB):
                    psO = [
                        ps.tile([65, 512], dt.float32, tag="ao", bufs=2, name=f"psO{h}")
                        for h in range(2)
                    ]
                    first_mm = [True, True]
                    qs = b * Tv + j * 512
                    for ip in range(0, 4 * j + 4, 2):
                        for h in range(2):
                            hs = slice(h * 64, (h + 1) * 64)
                            sc = ps.tile([128, 1024], dt.float32, tag="sc", bufs=2, name="sc")
                            for w in range(2):
                                i = ip + w
                                dd = i - 4 * j
                                kbase = b * Tv + i * 128
                                lo = max(dd, 0) * 128
                                nc.tensor.matmul(
                                    sc[:, w * 512 + lo: (w + 1) * 512],
                                    kT_sb[hs, kbase:kbase + 128],
                                    qT_sb[hs, qs + lo: qs + 512],
                                    start=True, stop=True,
                                )
                            eS = work.tile([128, 1024], dt.bfloat16, tag="es", bufs=3, name="eS")
                            dd0, dd1 = ip - 4 * j, ip + 1 - 4 * j
                            if dd1 <= 0:
                                nc.scalar.activation(eS[:], sc[:], AF.Exp, scale=0.125)
                            else:
                                lo0 = max(dd0, 0) * 128
                                nc.scalar.activation(eS[:, lo0:512], sc[:, lo0:512], AF.Exp, scale=0.125)
                                nc.scalar.activation(
                                    eS[:, 512 + dd1 * 128:1024], sc[:, 512 + dd1 * 128:1024],
                                    AF.Exp, scale=0.125,
                                )
                            for w in range(2):
                                dd = ip + w - 4 * j
                                if 0 <= dd <= 3:
                                    dcol = w * 512 + dd * 128
                                    nc.vector.tensor_tensor(
                                        eS[:, dcol:dcol + 128], eS[:, dcol:dcol + 128],
                                        mk_sb[:], ALU.mult,
                                    )
                            if debug_taps and b == 0 and j == 0 and ip == 0 and h == 0:
                                nc.sync.dma_start(dbg_es[:, 0:512], eS[:, 0:512])
                                nc.sync.dma_start(dbg_es[:, 640:1024], eS[:, 640:1024])
                            for w in range(2):
                                i = ip + w
                                dd = i - 4 * j
                                vsl = vaug[:, i * 130 + h * 65: i * 130 + (h + 1) * 65]
                                lo = max(dd, 0) * 128
                                nc.tensor.matmul(
                                    psO[h][:, lo:512], vsl, eS[:, w * 512 + lo: (w + 1) * 512],
                                    start=first_mm[h], stop=(dd == 3),
                                )
                                first_mm[h] = False
                    if debug_taps and b == 0 and j == 0:
                        ps_dbg = io.tile([65, 512], dt.float32, tag="psdbg", bufs=1)
                        nc.vector.tensor_copy(ps_dbg[:], psO[0][:])
                        nc.sync.dma_start(dbg_psO[:], ps_dbg[:])
                    for h in range(2):
                        drow = work.tile([1, 512], dt.float32, tag="drow", bufs=2, name="drow")
                        nc.vector.tensor_copy(drow[:], psO[h][64:65, :])
                        rrow = work.tile([1, 512], dt.float32, tag="rrow", bufs=2, name="rrow")
                        nc.vector.reciprocal_approx_fast(rrow[:], drow[:])
                        rrep = work.tile([64, 512], dt.float32, tag="rrep", bufs=2, name="rrep")
                        nc.gpsimd.partition_broadcast(rrep[:], rrow[:])
                        nc.vector.tensor_tensor(
                            aT_sb[h * 64:(h + 1) * 64, qs:qs + 512],
                            psO[h][0:64, :], rrep[:], ALU.mult,
                        )

            def phaseE(b):
                """Output projection (partial) for batch b."""
                if debug_taps:
                    nc.sync.dma_start(dbg_qT[:, b * Tv:(b + 1) * Tv], qT_sb[:, b * Tv:(b + 1) * Tv])
                    nc.sync.dma_start(dbg_kT[:, b * Tv:(b + 1) * Tv], kT_sb[:, b * Tv:(b + 1) * Tv])
                    nc.sync.dma_start(dbg_aT[:, b * Tv:(b + 1) * Tv], aT_sb[:, b * Tv:(b + 1) * Tv])
                for m in range(TPB):
                    g = b * TPB + m
                    psP = ps.tile([128, 1024], dt.float32, tag="sc", bufs=2, name="psP")
                    for nn in range(C // 512):
                        nc.tensor.matmul(
                            psP[:, nn * 512:(nn + 1) * 512],
                            aT_sb[:, g * 128:(g + 1) * 128],
                            wp_sb[:, nn * 512:(nn + 1) * 512],
                            start=True, stop=True,
                        )
                    osb = io.tile([128, C], dt.float32, tag="osb", bufs=2, name="osb")
                    nc.scalar.copy(osb[:, 0:512], psP[:, 0:512])
                    nc.vector.tensor_copy(osb[:, 512:1024], psP[:, 512:1024])
                    nc.sync.dma_start(outp_h[g * 128:(g + 1) * 128, :], osb[:])

            # software pipeline: emit next batch's projection before this
            # batch's attention so the PE stream never stalls on norm/rope
            phaseA(0)
            for b in range(Bv):
                phaseB(b)
                if b + 1 < Bv:
                    phaseA(b + 1)
                phaseC(b)
                phaseD(b)
                phaseE(b)

    nc.compile()
    return nc


def host_inputs(x, v1, W_qkv, b_qkv, W_proj, b_proj, lamb, Bv=B, Tv=T):
    """Shard + preprocess full inputs into per-core input maps."""
    NT = Bv * Tv
    TPB = Tv // 128
    lam = float(lamb)

    xT = np.ascontiguousarray(np.asarray(x, np.float32).reshape(NT, C).T).astype(ml_dtypes.bfloat16)

    # rope tables, token-major per 128-tile: cs[p, u*32+f] with position u*128+p
    pos = (np.arange(TPB)[:, None, None] * 128 + np.arange(128)[None, :, None]).astype(np.float32)
    inv_freq = (1.0 / ROPE_BASE ** (np.arange(0, D, 2, dtype=np.float32) / D))[None, None, :]
    ang = pos * inv_freq                      # [TPB, 128, 32]
    cos_t = np.cos(ang).transpose(1, 0, 2).reshape(128, TPB * 32)
    sin_t = np.sin(ang).transpose(1, 0, 2).reshape(128, TPB * 32)
    cs = np.concatenate([cos_t, cos_t, sin_t, sin_t], axis=1).astype(np.float32)
    cs = np.ascontiguousarray(cs)

    masku = np.triu(np.ones((128, 128), np.float32)).astype(ml_dtypes.bfloat16)
    ones = np.ones((1, 128), ml_dtypes.bfloat16)

    W_qkv = np.asarray(W_qkv, np.float32)
    b_qkv = np.asarray(b_qkv, np.float32)
    W_proj = np.asarray(W_proj, np.float32)
    v1 = np.asarray(v1, np.float32)

    in_maps = []
    for c in range(NCORES):
        r0 = c * HD
        Wq = W_qkv[r0:r0 + HD]
        Wk = W_qkv[C + r0:C + r0 + HD]
        Wv = W_qkv[2 * C + r0:2 * C + r0 + HD] * (1.0 - lam)
        wqkvT = np.ascontiguousarray(np.concatenate([Wq, Wk, Wv], axis=0).T).astype(ml_dtypes.bfloat16)
        brow = np.concatenate([
            b_qkv[r0:r0 + HD], b_qkv[C + r0:C + r0 + HD],
            b_qkv[2 * C + r0:2 * C + r0 + HD] * (1.0 - lam),
        ])[None, :].astype(np.float32)
        brow = np.ascontiguousarray(brow).astype(ml_dtypes.bfloat16)
        v1s = np.ascontiguousarray(
            (lam * v1[:, c * HPC:(c + 1) * HPC]).transpose(0, 2, 1, 3).reshape(NT, HD)
        )
        wp = np.ascontiguousarray(W_proj[:, r0:r0 + HD].T).astype(ml_dtypes.bfloat16)
        in_maps.append({
            "xT": xT, "wqkvT": wqkvT, "brow": brow, "ones": ones,
            "v1s": v1s, "cs": cs, "masku": masku, "wp": wp,
        })
    return in_maps


def host_gather(results, b_proj, Bv=B, Tv=T):
    NT = Bv * Tv
    out = np.zeros((NT, C), np.float32)
    for c in range(NCORES):
        out += results[c]["outp"]
    out += np.asarray(b_proj, np.float32)[None, :]
    out = out.reshape(Bv, Tv, C)
    value = np.empty((Bv, H, Tv, D), np.float32)
    for c in range(NCORES):
        value[:, c * HPC:(c + 1) * HPC] = (
            results[c]["val"].reshape(Bv, Tv, HPC, D).transpose(0, 2, 1, 3)
        )
    return out, value


_NC_CACHE = {}


def _get_module(Bv=B, Tv=T):
    key = (Bv, Tv)
    if key not in _NC_CACHE:
        _NC_CACHE[key] = build_module(Bv, Tv)
    return _NC_CACHE[key]


last_results = None


def kernel(x, v1, W_qkv, b_qkv, W_proj, b_proj, lamb, _trace=False):
    global last_results
    nc = _get_module()
    in_maps = host_inputs(x, v1, W_qkv, b_qkv, W_proj, b_proj, lamb)
    if _trace:
        _install_ntff_hook()
    res = run_bass_kernel_spmd(nc, in_maps, core_ids=list(range(NCORES)), trace=_trace)
    last_results = res
    return host_gather(res.results, b_proj)


def _install_ntff_hook():
    """Best-effort NTFF profiling hook for axon (used only when _trace=True)."""
    try:
        import types, sys
        if "antenv.axon_hooks" not in sys.modules:
            mod = types.ModuleType("antenv.axon_hooks")
            _h = [None]
            mod.set_axon_ntff_profile_hook = lambda h: _h.__setitem__(0, h)
            mod.get_axon_ntff_profile_hook = lambda: _h[0]
            sys.modules["antenv.axon_hooks"] = mod
        from antenv.axon_hooks import get_axon_ntff_profile_hook, set_axon_ntff_profile_hook
        if get_axon_ntff_profile_hook() is None:
            from trn_agent_boot.trn_boot import _ntff_profile_via_ctypes
            set_axon_ntff_profile_hook(_ntff_profile_via_ctypes("/opt/axon/libaxon_pjrt.so"))
    except Exception:
        pass


# revision 16
# speedup vs baseline: 1.0279x; 1.0279x over previous
"""Trainium2 Bass kernel for CausalSelfAttention (B=4, T=2048, C=1024, H=16, D=64).

Sharding: tensor-parallel over attention heads — 2 heads per core, 8 cores,
zero collectives. Each core computes QKV for its 2 heads (full token range),
runs causal attention, and produces a partial output projection
(its heads' columns of W_proj); the host sums the 8 partials and adds b_proj.
The mixed value tensor (an output of the module) is emitted per-core and
reassembled on the host.

Per-core dataflow (token-major QKV -> norm/rope -> PE transpose to dim-major
-> scores^T [k,q] -> exp (no max subtraction needed: qk-norm bounds scores)
-> attn@v with a ones-column to accumulate the softmax denominator ->
normalize -> output projection).
"""

import numpy as np
import ml_dtypes

import concourse.bass as bass
from concourse import bacc, mybir, tile, masks
from concourse.bass_utils import run_bass_kernel_spmd

dt = mybir.dt
AF = mybir.ActivationFunctionType
ALU = mybir.AluOpType

B, T, C, H, D = 4, 2048, 1024, 16, 64
NCORES = 8
HPC = H // NCORES          # heads per core
HD = HPC * D               # 128 head dims per core
ROPE_BASE = 10000.0
KC = C // 128              # contraction chunks for qkv proj


def build_module(Bv=B, Tv=T, debug_taps=False):
    """Build + compile the per-core Bass module. Identical on all cores (SPMD);
    only the input data differs per core."""
    NT = Bv * Tv
    TPB = Tv // 128        # token tiles per batch
    JPB = Tv // 512        # 512-wide q groups per batch
    QKW = 3 * HD           # 384 qkv output dims per core

    nc = bacc.Bacc("TRN2", target_bir_lowering=False, debug=False)
    if debug_taps:
        dbg_qT = nc.dram_tensor("dbg_qT", (128, NT), dt.bfloat16, kind="ExternalOutput").ap()
        dbg_kT = nc.dram_tensor("dbg_kT", (128, NT), dt.bfloat16, kind="ExternalOutput").ap()
        dbg_aT = nc.dram_tensor("dbg_aT", (128, NT), dt.bfloat16, kind="ExternalOutput").ap()
        dbg_es = nc.dram_tensor("dbg_es", (128, 1024), dt.bfloat16, kind="ExternalOutput").ap()
        dbg_psO = nc.dram_tensor("dbg_psO", (65, 512), dt.float32, kind="ExternalOutput").ap()

    xT_h = nc.dram_tensor("xT", (C, NT), dt.bfloat16, kind="ExternalInput").ap()
    wq_h = nc.dram_tensor("wqkvT", (C, QKW), dt.bfloat16, kind="ExternalInput").ap()
    br_h = nc.dram_tensor("brow", (1, QKW), dt.bfloat16, kind="ExternalInput").ap()
    on_h = nc.dram_tensor("ones", (1, 128), dt.bfloat16, kind="ExternalInput").ap()
    v1_h = nc.dram_tensor("v1s", (NT, HD), dt.float32, kind="ExternalInput").ap()
    cs_h = nc.dram_tensor("cs", (128, 4 * TPB * 32), dt.bfloat16, kind="ExternalInput").ap()
    mk_h = nc.dram_tensor("masku", (128, 128), dt.bfloat16, kind="ExternalInput").ap()
    wp_h = nc.dram_tensor("wp", (HD, C), dt.bfloat16, kind="ExternalInput").ap()
    outp_h = nc.dram_tensor("outp", (NT, C), dt.float32, kind="ExternalOutput").ap()
    val_h = nc.dram_tensor("val", (NT, HD), dt.float32, kind="ExternalOutput").ap()

    f32r = dt.float32r

    with tile.TileContext(nc) as tc:
        with (
            tc.tile_pool(name="const", bufs=1) as const,
            tc.tile_pool(name="work", bufs=1) as work,
            tc.tile_pool(name="io", bufs=1) as io,
            tc.tile_pool(name="ps", bufs=1, space="PSUM") as ps,
        ):
            # ---- constants / persistent tensors ----
            wq_sb = const.tile([128, KC * QKW], dt.bfloat16, name="wq_sb")
            for kc in range(KC):
                nc.sync.dma_start(
                    wq_sb[:, kc * QKW:(kc + 1) * QKW],
                    wq_h[kc * 128:(kc + 1) * 128, :],
                )
            br_sb = const.tile([1, QKW], dt.bfloat16, name="br_sb")
            nc.sync.dma_start(br_sb[:], br_h[:])
            on_sb = const.tile([1, 128], dt.bfloat16, name="on_sb")
            nc.sync.dma_start(on_sb[:], on_h[:])
            wp_sb = const.tile([HD, C], dt.bfloat16, name="wp_sb")
            nc.sync.dma_start(wp_sb[:], wp_h[:])
            cs_sb = const.tile([128, 4 * TPB * 32], dt.bfloat16, name="cs_sb")
            nc.sync.dma_start(cs_sb[:], cs_h[:])
            mk_sb = const.tile([128, 128], dt.bfloat16, name="mk_sb")
            nc.sync.dma_start(mk_sb[:], mk_h[:])
            id_sb = const.tile([128, 128], dt.bfloat16, name="id_sb")
            masks.make_identity(nc, id_sb[:])
            eps_sb = const.tile([128, 1], dt.float32, name="eps_sb")
            nc.vector.memset(eps_sb[:], 1e-6)

            qT_sb = const.tile([128, NT], dt.bfloat16, name="qT_sb")
            kT_sb = const.tile([128, NT], dt.bfloat16, name="kT_sb")
            aT_sb = const.tile([128, NT], dt.bfloat16, name="aT_sb")

            U = 2 * TPB  # rope groups (q tiles then k tiles)

            G = 2 * Tv // 64
            state = {}

            def phaseA_chunks(b):
                """QKV projection (token-major) for batch b, as per-m4 chunks."""
                qksb = work.tile([128, 2 * Tv], dt.float32, tag="qksb", bufs=2, name="qksb")
                vaug = work.tile([128, TPB * 130], dt.bfloat16, tag="vaug", bufs=2, name="vaug")
                nc.vector.memset(
                    vaug[:].rearrange("p (t h y) -> p t h y", h=2, y=65)[:, :, :, 64:65],
                    1.0,
                )
                state[("qksb", b)] = qksb
                state[("vaug", b)] = vaug

                def chunk(m4):
                    xt = io.tile([128, KC * 512], dt.bfloat16, tag="xt", bufs=2, name="xt")
                    g4 = b * TPB + m4 * 4
                    for kc in range(KC):
                        nc.gpsimd.dma_start(
                            xt[:, kc * 512:(kc + 1) * 512],
                            xT_h[kc * 128:(kc + 1) * 128, g4 * 128:(g4 + 4) * 128],
                        )
                    v1t = io.tile([128, 512], dt.float32, tag="v1t", bufs=2, name="v1t")
                    nc.gpsimd.dma_start(
                        v1t[:].rearrange("p (m d) -> p m d", d=HD),
                        v1_h[g4 * 128:(g4 + 4) * 128, :].rearrange("(m p) d -> p m d", p=128),
                    )
                    vmix4 = io.tile([128, 512], dt.float32, tag="vmix", bufs=2, name="vmix4")
                    for mi in range(4):
                        m = m4 * 4 + mi
                        ps_qkv = ps.tile([128, QKW], dt.float32, tag="mm", bufs=2, name="ps_qkv")
                        for kc in range(KC):
                            nc.tensor.matmul(
                                ps_qkv[:],
                                xt[:, kc * 512 + mi * 128: kc * 512 + (mi + 1) * 128],
                                wq_sb[:, kc * QKW:(kc + 1) * QKW],
                                start=(kc == 0),
                                stop=False,
                            )
                        nc.tensor.matmul(ps_qkv[:], on_sb[:], br_sb[:], start=False, stop=True)
                        nc.scalar.copy(
                            qksb[:].rearrange("p (s t) -> p s t", s=2)[:, :, m * 128:(m + 1) * 128],
                            ps_qkv[:, 0:256].rearrange("p (s u) -> p s u", s=2),
                        )
                        nc.vector.tensor_tensor(
                            vmix4[:, mi * HD:(mi + 1) * HD], ps_qkv[:, 256:384],
                            v1t[:, mi * HD:(mi + 1) * HD], ALU.add,
                        )
                    nc.gpsimd.dma_start(
                        val_h[g4 * 128:(g4 + 4) * 128, :].rearrange("(m p) d -> p m d", p=128),
                        vmix4[:].rearrange("p (m d) -> p m d", d=HD),
                    )
                    nc.vector.tensor_copy(
                        vaug[:, m4 * 520:(m4 + 1) * 520].rearrange("p (m h y) -> p m h y", m=4, y=65)[:, :, :, 0:64],
                        vmix4[:].rearrange("p (m h d) -> p m h d", h=2, d=64),
                    )

                return [(lambda mm: (lambda: chunk(mm)))(m4) for m4 in range(TPB // 4)]

            def phaseB(b):
                """rms-norm + rope for batch b."""
                qksb = state.pop(("qksb", b))
                sq = work.tile([128, 2 * Tv], dt.float32, tag="qkn", bufs=1, name="sq")
                nc.vector.tensor_tensor(sq[:], qksb[:], qksb[:], ALU.mult)
                ssum = work.tile([128, G], dt.float32, tag="ss", bufs=1, name="ssum")
                nc.vector.tensor_reduce(
                    ssum[:], sq[:].rearrange("p (g d) -> p g d", d=64),
                    mybir.AxisListType.X, ALU.add,
                )
                sfac = work.tile([128, G], dt.float32, tag="sf", bufs=1, name="sfac")
                nc.scalar.activation(sfac[:], ssum[:], AF.Sqrt, scale=1.0 / 64, bias=eps_sb[:])
                rfac = work.tile([128, G], dt.float32, tag="rf", bufs=1, name="rfac")
                nc.vector.reciprocal(rfac[:], sfac[:])
                qkn = work.tile([128, 2 * Tv], dt.bfloat16, tag="qkn2", bufs=1, name="qkn")
                nc.vector.tensor_tensor(
                    qkn[:].rearrange("p (g d) -> p g d", d=64),
                    qksb[:].rearrange("p (g d) -> p g d", d=64),
                    rfac[:].broadcast_to([128, G, 64]),
                    ALU.mult,
                )
                rbf = work.tile([128, 2 * Tv], dt.bfloat16, tag="rbf", bufs=1, name="rbf")
                qk4 = qkn[:].rearrange("p (u h d) -> p u h d", h=2, d=64)
                rb4 = rbf[:].rearrange("p (u h d) -> p u h d", h=2, d=64)
                cosv = cs_sb[:, 0:U * 32].rearrange("p (u f) -> p u f", f=32)
                sinv = cs_sb[:, 2 * TPB * 32:2 * TPB * 32 + U * 32].rearrange("p (u f) -> p u f", f=32)
                for h in range(2):
                    x1 = qk4[:, :, h, 0:32]
                    x2 = qk4[:, :, h, 32:64]
                    t1 = work.tile([128, U * 32], dt.bfloat16, tag="tmp", bufs=2, name="t1")
                    t2 = work.tile([128, U * 32], dt.bfloat16, tag="tmp", bufs=2, name="t2")
                    t1v = t1[:].rearrange("p (u f) -> p u f", f=32)
                    t2v = t2[:].rearrange("p (u f) -> p u f", f=32)
                    nc.vector.tensor_tensor(t1v, x1, cosv, ALU.mult)
                    nc.vector.tensor_tensor(t2v, x2, sinv, ALU.mult)
                    nc.vector.tensor_tensor(rb4[:, :, h, 0:32], t1v, t2v, ALU.add)
                    t3 = work.tile([128, U * 32], dt.bfloat16, tag="tmp", bufs=2, name="t3")
                    t4 = work.tile([128, U * 32], dt.bfloat16, tag="tmp", bufs=2, name="t4")
                    t3v = t3[:].rearrange("p (u f) -> p u f", f=32)
                    t4v = t4[:].rearrange("p (u f) -> p u f", f=32)
                    nc.vector.tensor_tensor(t3v, x2, cosv, ALU.mult)
                    nc.vector.tensor_tensor(t4v, x1, sinv, ALU.mult)
                    nc.vector.tensor_tensor(rb4[:, :, h, 32:64], t3v, t4v, ALU.subtract)
                state[("rbf", b)] = rbf

            def phaseC(b):
                """PE transpose to dim-major."""
                rbf = state.pop(("rbf", b))
                for half, dst in ((0, qT_sb), (1, kT_sb)):
                    for p4 in range(TPB // 4):  # noqa: loop kept flat
                        tp = ps.tile([128, 512], dt.bfloat16, tag="mm", bufs=2, name="tp")
                        for t4 in range(4):
                            m = p4 * 4 + t4
                            nc.tensor.transpose(
                                tp[:, t4 * 128:(t4 + 1) * 128],
                                rbf[:, half * Tv + m * 128: half * Tv + (m + 1) * 128],
                                id_sb[:],
                            )
                        nc.vector.tensor_copy(dst[:, b * Tv + p4 * 512: b * Tv + (p4 + 1) * 512], tp[:])

            def phaseD_chunks(b):
                """Causal attention for batch b, as per-(j, ip) chunks plus a
                normalize chunk per j."""
                vaug = state.pop(("vaug", b))
                chunks = []
                for j in range(JPB):
                    psO = [
                        ps.tile([65, 512], dt.float32, tag="ao", bufs=2, name=f"psO{h}")
                        for h in range(2)
                    ]
                    first_mm = [True, True]
                    qs = b * Tv + j * 512

                    def ip_chunk(j, ip, psO, first_mm, qs):
                        for h in range(2):
                            hs = slice(h * 64, (h + 1) * 64)
                            sc = ps.tile([128, 1024], dt.float32, tag="sc", bufs=2, name="sc")
                            for w in range(2):
                                i = ip + w
                                dd = i - 4 * j
                                kbase = b * Tv + i * 128
                                lo = max(dd, 0) * 128
                                nc.tensor.matmul(
                                    sc[:, w * 512 + lo: (w + 1) * 512],
                                    kT_sb[hs, kbase:kbase + 128],
                                    qT_sb[hs, qs + lo: qs + 512],
                                    start=True, stop=True,
                                )
                            eS = work.tile([128, 1024], dt.bfloat16, tag="es", bufs=4, name="eS")
                            dd0, dd1 = ip - 4 * j, ip + 1 - 4 * j
                            if dd1 <= 0:
                                nc.scalar.activation(eS[:], sc[:], AF.Exp, scale=0.125)
                            else:
                                lo0 = max(dd0, 0) * 128
                                nc.scalar.activation(eS[:, lo0:512], sc[:, lo0:512], AF.Exp, scale=0.125)
                                nc.scalar.activation(
                                    eS[:, 512 + dd1 * 128:1024], sc[:, 512 + dd1 * 128:1024],
                                    AF.Exp, scale=0.125,
                                )
                            for w in range(2):
                                dd = ip + w - 4 * j
                                if 0 <= dd <= 3:
                                    dcol = w * 512 + dd * 128
                                    nc.vector.tensor_tensor(
                                        eS[:, dcol:dcol + 128], eS[:, dcol:dcol + 128],
                                        mk_sb[:], ALU.mult,
                                    )
                            if debug_taps and b == 0 and j == 0 and ip == 0 and h == 0:
                                nc.sync.dma_start(dbg_es[:, 0:512], eS[:, 0:512])
                                nc.sync.dma_start(dbg_es[:, 640:1024], eS[:, 640:1024])
                            for w in range(2):
                                i = ip + w
                                dd = i - 4 * j
                                vsl = vaug[:, i * 130 + h * 65: i * 130 + (h + 1) * 65]
                                lo = max(dd, 0) * 128
                                nc.tensor.matmul(
                                    psO[h][:, lo:512], vsl, eS[:, w * 512 + lo: (w + 1) * 512],
                                    start=first_mm[h], stop=(dd == 3),
                                )
                                first_mm[h] = False
                    def norm_chunk(j, psO, qs):
                        if debug_taps and b == 0 and j == 0:
                            ps_dbg = io.tile([65, 512], dt.float32, tag="psdbg", bufs=1)
                            nc.vector.tensor_copy(ps_dbg[:], psO[0][:])
                            nc.sync.dma_start(dbg_psO[:], ps_dbg[:])
                        for h in range(2):
                            drow = work.tile([1, 512], dt.float32, tag="drow", bufs=2, name="drow")
                            nc.vector.tensor_copy(drow[:], psO[h][64:65, :])
                            rrow = work.tile([1, 512], dt.float32, tag="rrow", bufs=2, name="rrow")
                            nc.vector.reciprocal_approx_fast(rrow[:], drow[:])
                            rrep = work.tile([64, 512], dt.float32, tag="rrep", bufs=2, name="rrep")
                            nc.gpsimd.partition_broadcast(rrep[:], rrow[:])
                            nc.vector.tensor_tensor(
                                aT_sb[h * 64:(h + 1) * 64, qs:qs + 512],
                                psO[h][0:64, :], rrep[:], ALU.mult,
                            )

                    for ip in range(0, 4 * j + 4, 2):
                        chunks.append((lambda jj, pp, oo, ff, qq: (lambda: ip_chunk(jj, pp, oo, ff, qq)))(j, ip, psO, first_mm, qs))
                    chunks.append((lambda jj, oo, qq: (lambda: norm_chunk(jj, oo, qq)))(j, psO, qs))
                return chunks

            def phaseE_chunks(b):
                """Output projection (partial) for batch b, per-m4 chunks."""
                if debug_taps:
                    nc.sync.dma_start(dbg_qT[:, b * Tv:(b + 1) * Tv], qT_sb[:, b * Tv:(b + 1) * Tv])
                    nc.sync.dma_start(dbg_kT[:, b * Tv:(b + 1) * Tv], kT_sb[:, b * Tv:(b + 1) * Tv])
                    nc.sync.dma_start(dbg_aT[:, b * Tv:(b + 1) * Tv], aT_sb[:, b * Tv:(b + 1) * Tv])

                def e_chunk(m):
                    g = b * TPB + m
                    psP = ps.tile([128, 1024], dt.float32, tag="sc", bufs=2, name="psP")
                    for nn in range(C // 512):
                        nc.tensor.matmul(
                            psP[:, nn * 512:(nn + 1) * 512],
                            aT_sb[:, g * 128:(g + 1) * 128],
                            wp_sb[:, nn * 512:(nn + 1) * 512],
                            start=True, stop=True,
                        )
                    osb = io.tile([128, C], dt.float32, tag="osb", bufs=2, name="osb")
                    nc.scalar.copy(osb[:, 0:512], psP[:, 0:512])
                    nc.vector.tensor_copy(osb[:, 512:1024], psP[:, 512:1024])
                    nc.sync.dma_start(outp_h[g * 128:(g + 1) * 128, :], osb[:])

                return [(lambda mm: (lambda: e_chunk(mm)))(m) for m in range(TPB)]

            # Software-pipelined, interleaved emission. During batch b's
            # attention (exp-latency bound on ACT), inject next batch's QKV
            # matmul chunks and previous batch's out-proj chunks into the PE
            # stream so the tensor engine stays dense (keeps HAM at K=8/8).
            for chunk in phaseA_chunks(0):
                chunk()
            filler = []
            for b in range(Bv):
                phaseB(b)
                phaseC(b)
                if b + 1 < Bv:
                    filler.extend(phaseA_chunks(b + 1))
                d_chunks = phaseD_chunks(b)
                n_d = len(d_chunks)
                n_f = len(filler)
                fi = 0
                for ci, ch in enumerate(d_chunks):
                    ch()
                    want = (ci + 1) * n_f // n_d
                    while fi < want:
                        filler[fi]()
                        fi += 1
                filler = list(phaseE_chunks(b))
            for ch in filler:
                ch()

    nc.compile()
    return nc


def host_inputs(x, v1, W_qkv, b_qkv, W_proj, b_proj, lamb, Bv=B, Tv=T):
    """Shard + preprocess full inputs into per-core input maps."""
    NT = Bv * Tv
    TPB = Tv // 128
    lam = float(lamb)

    xT = np.ascontiguousarray(np.asarray(x, np.float32).reshape(NT, C).T).astype(ml_dtypes.bfloat16)

    # rope tables, token-major per 128-tile: cs[p, u*32+f] with position u*128+p
    pos = (np.arange(TPB)[:, None, None] * 128 + np.arange(128)[None, :, None]).astype(np.float32)
    inv_freq = (1.0 / ROPE_BASE ** (np.arange(0, D, 2, dtype=np.float32) / D))[None, None, :]
    ang = pos * inv_freq                      # [TPB, 128, 32]
    cos_t = np.cos(ang).transpose(1, 0, 2).reshape(128, TPB * 32)
    sin_t = np.sin(ang).transpose(1, 0, 2).reshape(128, TPB * 32)
    cs = np.concatenate([cos_t, cos_t, sin_t, sin_t], axis=1).astype(ml_dtypes.bfloat16)
    cs = np.ascontiguousarray(cs)

    masku = np.triu(np.ones((128, 128), np.float32)).astype(ml_dtypes.bfloat16)
    ones = np.ones((1, 128), ml_dtypes.bfloat16)

    W_qkv = np.asarray(W_qkv, np.float32)
    b_qkv = np.asarray(b_qkv, np.float32)
    W_proj = np.asarray(W_proj, np.float32)
    v1 = np.asarray(v1, np.float32)

    in_maps = []
    for c in range(NCORES):
        r0 = c * HD
        Wq = W_qkv[r0:r0 + HD]
        Wk = W_qkv[C + r0:C + r0 + HD]
        Wv = W_qkv[2 * C + r0:2 * C + r0 + HD] * (1.0 - lam)
        wqkvT = np.ascontiguousarray(np.concatenate([Wq, Wk, Wv], axis=0).T).astype(ml_dtypes.bfloat16)
        brow = np.concatenate([
            b_qkv[r0:r0 + HD], b_qkv[C + r0:C + r0 + HD],
            b_qkv[2 * C + r0:2 * C + r0 + HD] * (1.0 - lam),
        ])[None, :].astype(np.float32)
        brow = np.ascontiguousarray(brow).astype(ml_dtypes.bfloat16)
        v1s = np.ascontiguousarray(
            (lam * v1[:, c * HPC:(c + 1) * HPC]).transpose(0, 2, 1, 3).reshape(NT, HD)
        )
        wp = np.ascontiguousarray(W_proj[:, r0:r0 + HD].T).astype(ml_dtypes.bfloat16)
        in_maps.append({
            "xT": xT, "wqkvT": wqkvT, "brow": brow, "ones": ones,
            "v1s": v1s, "cs": cs, "masku": masku, "wp": wp,
        })
    return in_maps


def host_gather(results, b_proj, Bv=B, Tv=T):
    NT = Bv * Tv
    out = np.zeros((NT, C), np.float32)
    for c in range(NCORES):
        out += results[c]["outp"]
    out += np.asarray(b_proj, np.float32)[None, :]
    out = out.reshape(Bv, Tv, C)
    value = np.empty((Bv, H, Tv, D), np.float32)
    for c in range(NCORES):
        value[:, c * HPC:(c + 1) * HPC] = (
            results[c]["val"].reshape(Bv, Tv, HPC, D).transpose(0, 2, 1, 3)
        )
    return out, value


_NC_CACHE = {}


def _get_module(Bv=B, Tv=T):
    key = (Bv, Tv)
    if key not in _NC_CACHE:
        _NC_CACHE[key] = build_module(Bv, Tv)
    return _NC_CACHE[key]


last_results = None


def kernel(x, v1, W_qkv, b_qkv, W_proj, b_proj, lamb, _trace=False):
    global last_results
    nc = _get_module()
    in_maps = host_inputs(x, v1, W_qkv, b_qkv, W_proj, b_proj, lamb)
    if _trace:
        _install_ntff_hook()
    res = run_bass_kernel_spmd(nc, in_maps, core_ids=list(range(NCORES)), trace=_trace)
    last_results = res
    return host_gather(res.results, b_proj)


def _install_ntff_hook():
    """Best-effort NTFF profiling hook for axon (used only when _trace=True)."""
    try:
        import types, sys
        if "antenv.axon_hooks" not in sys.modules:
            mod = types.ModuleType("antenv.axon_hooks")
            _h = [None]
            mod.set_axon_ntff_profile_hook = lambda h: _h.__setitem__(0, h)
            mod.get_axon_ntff_profile_hook = lambda: _h[0]
            sys.modules["antenv.axon_hooks"] = mod
        from antenv.axon_hooks import get_axon_ntff_profile_hook, set_axon_ntff_profile_hook
        if get_axon_ntff_profile_hook() is None:
            from trn_agent_boot.trn_boot import _ntff_profile_via_ctypes
            set_axon_ntff_profile_hook(_ntff_profile_via_ctypes("/opt/axon/libaxon_pjrt.so"))
    except Exception:
        pass


# revision 17
# speedup vs baseline: 1.0398x; 1.0116x over previous
"""Trainium2 Bass kernel for CausalSelfAttention (B=4, T=2048, C=1024, H=16, D=64).

Sharding: tensor-parallel over attention heads — 2 heads per core, 8 cores,
zero collectives. Each core computes QKV for its 2 heads (full token range),
runs causal attention, and produces a partial output projection
(its heads' columns of W_proj); the host sums the 8 partials and adds b_proj.
The mixed value tensor (an output of the module) is emitted per-core and
reassembled on the host.

Per-core dataflow (token-major QKV -> norm/rope -> PE transpose to dim-major
-> scores^T [k,q] -> exp (no max subtraction needed: qk-norm bounds scores)
-> attn@v with a ones-column to accumulate the softmax denominator ->
normalize -> output projection).
"""

import numpy as np
import ml_dtypes

import concourse.bass as bass
from concourse import bacc, mybir, tile, masks
from concourse.bass_utils import run_bass_kernel_spmd

dt = mybir.dt
AF = mybir.ActivationFunctionType
ALU = mybir.AluOpType

B, T, C, H, D = 4, 2048, 1024, 16, 64
NCORES = 8
HPC = H // NCORES          # heads per core
HD = HPC * D               # 128 head dims per core
ROPE_BASE = 10000.0
KC = C // 128              # contraction chunks for qkv proj


def build_module(Bv=B, Tv=T, debug_taps=False):
    """Build + compile the per-core Bass module. Identical on all cores (SPMD);
    only the input data differs per core."""
    NT = Bv * Tv
    TPB = Tv // 128        # token tiles per batch
    JPB = Tv // 512        # 512-wide q groups per batch
    QKW = 3 * HD           # 384 qkv output dims per core

    nc = bacc.Bacc("TRN2", target_bir_lowering=False, debug=False)
    if debug_taps:
        dbg_qT = nc.dram_tensor("dbg_qT", (128, NT), dt.bfloat16, kind="ExternalOutput").ap()
        dbg_kT = nc.dram_tensor("dbg_kT", (128, NT), dt.bfloat16, kind="ExternalOutput").ap()
        dbg_aT = nc.dram_tensor("dbg_aT", (128, NT), dt.bfloat16, kind="ExternalOutput").ap()
        dbg_es = nc.dram_tensor("dbg_es", (128, 1024), dt.bfloat16, kind="ExternalOutput").ap()
        dbg_psO = nc.dram_tensor("dbg_psO", (65, 512), dt.float32, kind="ExternalOutput").ap()

    xT_h = nc.dram_tensor("xT", (C, NT), dt.bfloat16, kind="ExternalInput").ap()
    wq_h = nc.dram_tensor("wqkvT", (C, QKW), dt.bfloat16, kind="ExternalInput").ap()
    br_h = nc.dram_tensor("brow", (1, QKW), dt.bfloat16, kind="ExternalInput").ap()
    on_h = nc.dram_tensor("ones", (1, 128), dt.bfloat16, kind="ExternalInput").ap()
    v1_h = nc.dram_tensor("v1s", (NT, HD), dt.float32, kind="ExternalInput").ap()
    cs_h = nc.dram_tensor("cs", (128, 4 * TPB * 32), dt.bfloat16, kind="ExternalInput").ap()
    mk_h = nc.dram_tensor("masku", (128, 128), dt.bfloat16, kind="ExternalInput").ap()
    wp_h = nc.dram_tensor("wp", (HD, C), dt.bfloat16, kind="ExternalInput").ap()
    outp_h = nc.dram_tensor("outp", (NT, C), dt.float32, kind="ExternalOutput").ap()
    val_h = nc.dram_tensor("val", (NT, HD), dt.float32, kind="ExternalOutput").ap()

    f32r = dt.float32r

    with tile.TileContext(nc) as tc:
        with (
            tc.tile_pool(name="const", bufs=1) as const,
            tc.tile_pool(name="work", bufs=1) as work,
            tc.tile_pool(name="io", bufs=1) as io,
            tc.tile_pool(name="ps", bufs=1, space="PSUM") as ps,
        ):
            # ---- constants / persistent tensors ----
            wq_sb = const.tile([128, KC * QKW], dt.bfloat16, name="wq_sb")
            for kc in range(KC):
                nc.sync.dma_start(
                    wq_sb[:, kc * QKW:(kc + 1) * QKW],
                    wq_h[kc * 128:(kc + 1) * 128, :],
                )
            br_sb = const.tile([1, QKW], dt.bfloat16, name="br_sb")
            nc.sync.dma_start(br_sb[:], br_h[:])
            on_sb = const.tile([1, 128], dt.bfloat16, name="on_sb")
            nc.sync.dma_start(on_sb[:], on_h[:])
            wp_sb = const.tile([HD, C], dt.bfloat16, name="wp_sb")
            nc.sync.dma_start(wp_sb[:], wp_h[:])
            cs_sb = const.tile([128, 4 * TPB * 32], dt.bfloat16, name="cs_sb")
            nc.sync.dma_start(cs_sb[:], cs_h[:])
            mk_sb = const.tile([128, 128], dt.bfloat16, name="mk_sb")
            nc.sync.dma_start(mk_sb[:], mk_h[:])
            id_sb = const.tile([128, 128], dt.bfloat16, name="id_sb")
            masks.make_identity(nc, id_sb[:])
            eps_sb = const.tile([128, 1], dt.float32, name="eps_sb")
            nc.vector.memset(eps_sb[:], 1e-6)

            qT_sb = const.tile([128, NT], dt.bfloat16, name="qT_sb")
            kT_sb = const.tile([128, NT], dt.bfloat16, name="kT_sb")
            aT_sb = const.tile([128, NT], dt.bfloat16, name="aT_sb")

            U = 2 * TPB  # rope groups (q tiles then k tiles)

            G = 2 * Tv // 64
            state = {}

            def phaseA_chunks(b):
                """QKV projection (token-major) for batch b, as per-m4 chunks."""
                qksb = work.tile([128, 2 * Tv], dt.float32, tag="qksb", bufs=2, name="qksb")
                vaug = work.tile([128, TPB * 130], dt.bfloat16, tag="vaug", bufs=2, name="vaug")
                nc.vector.memset(
                    vaug[:].rearrange("p (t h y) -> p t h y", h=2, y=65)[:, :, :, 64:65],
                    1.0,
                )
                state[("qksb", b)] = qksb
                state[("vaug", b)] = vaug

                def chunk(m4):
                    xt = io.tile([128, KC * 512], dt.bfloat16, tag="xt", bufs=2, name="xt")
                    g4 = b * TPB + m4 * 4
                    for kc in range(KC):
                        nc.sync.dma_start(
                            xt[:, kc * 512:(kc + 1) * 512],
                            xT_h[kc * 128:(kc + 1) * 128, g4 * 128:(g4 + 4) * 128],
                        )
                    v1t = io.tile([128, 512], dt.float32, tag="v1t", bufs=2, name="v1t")
                    nc.sync.dma_start(
                        v1t[:].rearrange("p (m d) -> p m d", d=HD),
                        v1_h[g4 * 128:(g4 + 4) * 128, :].rearrange("(m p) d -> p m d", p=128),
                    )
                    vmix4 = io.tile([128, 512], dt.float32, tag="vmix", bufs=2, name="vmix4")
                    for mi in range(4):
                        m = m4 * 4 + mi
                        ps_qkv = ps.tile([128, QKW], dt.float32, tag="mm", bufs=2, name="ps_qkv")
                        for kc in range(KC):
                            nc.tensor.matmul(
                                ps_qkv[:],
                                xt[:, kc * 512 + mi * 128: kc * 512 + (mi + 1) * 128],
                                wq_sb[:, kc * QKW:(kc + 1) * QKW],
                                start=(kc == 0),
                                stop=False,
                            )
                        nc.tensor.matmul(ps_qkv[:], on_sb[:], br_sb[:], start=False, stop=True)
                        nc.scalar.copy(
                            qksb[:].rearrange("p (s t) -> p s t", s=2)[:, :, m * 128:(m + 1) * 128],
                            ps_qkv[:, 0:256].rearrange("p (s u) -> p s u", s=2),
                        )
                        nc.vector.tensor_tensor(
                            vmix4[:, mi * HD:(mi + 1) * HD], ps_qkv[:, 256:384],
                            v1t[:, mi * HD:(mi + 1) * HD], ALU.add,
                        )
                    nc.sync.dma_start(
                        val_h[g4 * 128:(g4 + 4) * 128, :].rearrange("(m p) d -> p m d", p=128),
                        vmix4[:].rearrange("p (m d) -> p m d", d=HD),
                    )
                    nc.vector.tensor_copy(
                        vaug[:, m4 * 520:(m4 + 1) * 520].rearrange("p (m h y) -> p m h y", m=4, y=65)[:, :, :, 0:64],
                        vmix4[:].rearrange("p (m h d) -> p m h d", h=2, d=64),
                    )

                return [(lambda mm: (lambda: chunk(mm)))(m4) for m4 in range(TPB // 4)]

            def phaseB(b):
                """rms-norm + rope for batch b."""
                qksb = state.pop(("qksb", b))
                sq = work.tile([128, 2 * Tv], dt.float32, tag="qkn", bufs=1, name="sq")
                nc.vector.tensor_tensor(sq[:], qksb[:], qksb[:], ALU.mult)
                ssum = work.tile([128, G], dt.float32, tag="ss", bufs=1, name="ssum")
                nc.vector.tensor_reduce(
                    ssum[:], sq[:].rearrange("p (g d) -> p g d", d=64),
                    mybir.AxisListType.X, ALU.add,
                )
                sfac = work.tile([128, G], dt.float32, tag="sf", bufs=1, name="sfac")
                nc.scalar.activation(sfac[:], ssum[:], AF.Sqrt, scale=1.0 / 64, bias=eps_sb[:])
                rfac = work.tile([128, G], dt.float32, tag="rf", bufs=1, name="rfac")
                nc.vector.reciprocal(rfac[:], sfac[:])
                qkn = work.tile([128, 2 * Tv], dt.bfloat16, tag="qkn2", bufs=1, name="qkn")
                nc.vector.tensor_tensor(
                    qkn[:].rearrange("p (g d) -> p g d", d=64),
                    qksb[:].rearrange("p (g d) -> p g d", d=64),
                    rfac[:].broadcast_to([128, G, 64]),
                    ALU.mult,
                )
                rbf = work.tile([128, 2 * Tv], dt.bfloat16, tag="rbf", bufs=1, name="rbf")
                qk4 = qkn[:].rearrange("p (u h d) -> p u h d", h=2, d=64)
                rb4 = rbf[:].rearrange("p (u h d) -> p u h d", h=2, d=64)
                cosv = cs_sb[:, 0:U * 32].rearrange("p (u f) -> p u f", f=32)
                sinv = cs_sb[:, 2 * TPB * 32:2 * TPB * 32 + U * 32].rearrange("p (u f) -> p u f", f=32)
                for h in range(2):
                    x1 = qk4[:, :, h, 0:32]
                    x2 = qk4[:, :, h, 32:64]
                    t1 = work.tile([128, U * 32], dt.bfloat16, tag="tmp", bufs=2, name="t1")
                    t2 = work.tile([128, U * 32], dt.bfloat16, tag="tmp", bufs=2, name="t2")
                    t1v = t1[:].rearrange("p (u f) -> p u f", f=32)
                    t2v = t2[:].rearrange("p (u f) -> p u f", f=32)
                    nc.vector.tensor_tensor(t1v, x1, cosv, ALU.mult)
                    nc.vector.tensor_tensor(t2v, x2, sinv, ALU.mult)
                    nc.vector.tensor_tensor(rb4[:, :, h, 0:32], t1v, t2v, ALU.add)
                    t3 = work.tile([128, U * 32], dt.bfloat16, tag="tmp", bufs=2, name="t3")
                    t4 = work.tile([128, U * 32], dt.bfloat16, tag="tmp", bufs=2, name="t4")
                    t3v = t3[:].rearrange("p (u f) -> p u f", f=32)
                    t4v = t4[:].rearrange("p (u f) -> p u f", f=32)
                    nc.vector.tensor_tensor(t3v, x2, cosv, ALU.mult)
                    nc.vector.tensor_tensor(t4v, x1, sinv, ALU.mult)
                    nc.vector.tensor_tensor(rb4[:, :, h, 32:64], t3v, t4v, ALU.subtract)
                state[("rbf", b)] = rbf

            def phaseC(b):
                """PE transpose to dim-major."""
                rbf = state.pop(("rbf", b))
                for half, dst in ((0, qT_sb), (1, kT_sb)):
                    for p4 in range(TPB // 4):  # noqa: loop kept flat
                        tp = ps.tile([128, 512], dt.bfloat16, tag="mm", bufs=2, name="tp")
                        for t4 in range(4):
                            m = p4 * 4 + t4
                            nc.tensor.transpose(
                                tp[:, t4 * 128:(t4 + 1) * 128],
                                rbf[:, half * Tv + m * 128: half * Tv + (m + 1) * 128],
                                id_sb[:],
                            )
                        nc.vector.tensor_copy(dst[:, b * Tv + p4 * 512: b * Tv + (p4 + 1) * 512], tp[:])

            def phaseD_chunks(b):
                """Causal attention for batch b, as per-(j, ip) chunks plus a
                normalize chunk per j."""
                vaug = state.pop(("vaug", b))
                chunks = []
                for j in range(JPB):
                    psO = [
                        ps.tile([65, 512], dt.float32, tag="ao", bufs=2, name=f"psO{h}")
                        for h in range(2)
                    ]
                    first_mm = [True, True]
                    qs = b * Tv + j * 512

                    def ip_chunk(j, ip, psO, first_mm, qs):
                        for h in range(2):
                            hs = slice(h * 64, (h + 1) * 64)
                            sc = ps.tile([128, 1024], dt.float32, tag="sc", bufs=2, name="sc")
                            for w in range(2):
                                i = ip + w
                                dd = i - 4 * j
                                kbase = b * Tv + i * 128
                                lo = max(dd, 0) * 128
                                nc.tensor.matmul(
                                    sc[:, w * 512 + lo: (w + 1) * 512],
                                    kT_sb[hs, kbase:kbase + 128],
                                    qT_sb[hs, qs + lo: qs + 512],
                                    start=True, stop=True,
                                )
                            eS = work.tile([128, 1024], dt.bfloat16, tag="es", bufs=4, name="eS")
                            dd0, dd1 = ip - 4 * j, ip + 1 - 4 * j
                            if dd1 <= 0:
                                nc.scalar.activation(eS[:], sc[:], AF.Exp, scale=0.125)
                            else:
                                lo0 = max(dd0, 0) * 128
                                nc.scalar.activation(eS[:, lo0:512], sc[:, lo0:512], AF.Exp, scale=0.125)
                                nc.scalar.activation(
                                    eS[:, 512 + dd1 * 128:1024], sc[:, 512 + dd1 * 128:1024],
                                    AF.Exp, scale=0.125,
                                )
                            for w in range(2):
                                dd = ip + w - 4 * j
                                if 0 <= dd <= 3:
                                    dcol = w * 512 + dd * 128
                                    nc.vector.tensor_tensor(
                                        eS[:, dcol:dcol + 128], eS[:, dcol:dcol + 128],
                                        mk_sb[:], ALU.mult,
                                    )
                            if debug_taps and b == 0 and j == 0 and ip == 0 and h == 0:
                                nc.sync.dma_start(dbg_es[:, 0:512], eS[:, 0:512])
                                nc.sync.dma_start(dbg_es[:, 640:1024], eS[:, 640:1024])
                            for w in range(2):
                                i = ip + w
                                dd = i - 4 * j
                                vsl = vaug[:, i * 130 + h * 65: i * 130 + (h + 1) * 65]
                                lo = max(dd, 0) * 128
                                nc.tensor.matmul(
                                    psO[h][:, lo:512], vsl, eS[:, w * 512 + lo: (w + 1) * 512],
                                    start=first_mm[h], stop=(dd == 3),
                                )
                                first_mm[h] = False
                    def norm_chunk(j, psO, qs):
                        if debug_taps and b == 0 and j == 0:
                            ps_dbg = io.tile([65, 512], dt.float32, tag="psdbg", bufs=1)
                            nc.vector.tensor_copy(ps_dbg[:], psO[0][:])
                            nc.sync.dma_start(dbg_psO[:], ps_dbg[:])
                        for h in range(2):
                            drow = work.tile([1, 512], dt.float32, tag="drow", bufs=2, name="drow")
                            nc.vector.tensor_copy(drow[:], psO[h][64:65, :])
                            rrow = work.tile([1, 512], dt.float32, tag="rrow", bufs=2, name="rrow")
                            nc.vector.reciprocal_approx_fast(rrow[:], drow[:])
                            rrep = work.tile([64, 512], dt.float32, tag="rrep", bufs=2, name="rrep")
                            nc.gpsimd.partition_broadcast(rrep[:], rrow[:])
                            nc.vector.tensor_tensor(
                                aT_sb[h * 64:(h + 1) * 64, qs:qs + 512],
                                psO[h][0:64, :], rrep[:], ALU.mult,
                            )

                    for ip in range(0, 4 * j + 4, 2):
                        chunks.append((lambda jj, pp, oo, ff, qq: (lambda: ip_chunk(jj, pp, oo, ff, qq)))(j, ip, psO, first_mm, qs))
                    chunks.append((lambda jj, oo, qq: (lambda: norm_chunk(jj, oo, qq)))(j, psO, qs))
                return chunks

            def phaseE_chunks(b):
                """Output projection (partial) for batch b, per-m4 chunks."""
                if debug_taps:
                    nc.sync.dma_start(dbg_qT[:, b * Tv:(b + 1) * Tv], qT_sb[:, b * Tv:(b + 1) * Tv])
                    nc.sync.dma_start(dbg_kT[:, b * Tv:(b + 1) * Tv], kT_sb[:, b * Tv:(b + 1) * Tv])
                    nc.sync.dma_start(dbg_aT[:, b * Tv:(b + 1) * Tv], aT_sb[:, b * Tv:(b + 1) * Tv])

                def e_chunk(m):
                    g = b * TPB + m
                    psP = ps.tile([128, 1024], dt.float32, tag="sc", bufs=2, name="psP")
                    for nn in range(C // 512):
                        nc.tensor.matmul(
                            psP[:, nn * 512:(nn + 1) * 512],
                            aT_sb[:, g * 128:(g + 1) * 128],
                            wp_sb[:, nn * 512:(nn + 1) * 512],
                            start=True, stop=True,
                        )
                    osb = io.tile([128, C], dt.float32, tag="osb", bufs=2, name="osb")
                    nc.scalar.copy(osb[:, 0:512], psP[:, 0:512])
                    nc.vector.tensor_copy(osb[:, 512:1024], psP[:, 512:1024])
                    nc.gpsimd.dma_start(outp_h[g * 128:(g + 1) * 128, :], osb[:])

                return [(lambda mm: (lambda: e_chunk(mm)))(m) for m in range(TPB)]

            # Software-pipelined, interleaved emission. During batch b's
            # attention (exp-latency bound on ACT), inject next batch's QKV
            # matmul chunks and previous batch's out-proj chunks into the PE
            # stream so the tensor engine stays dense (keeps HAM at K=8/8).
            for chunk in phaseA_chunks(0):
                chunk()
            filler = []
            for b in range(Bv):
                phaseB(b)
                # next batch's projection fills the PE while norm/rope runs on DVE
                if b + 1 < Bv:
                    for ch in phaseA_chunks(b + 1):
                        ch()
                phaseC(b)
                d_chunks = phaseD_chunks(b)
                n_d = len(d_chunks)
                n_f = len(filler)
                fi = 0
                for ci, ch in enumerate(d_chunks):
                    ch()
                    want = (ci + 1) * n_f // n_d
                    while fi < want:
                        filler[fi]()
                        fi += 1
                filler = list(phaseE_chunks(b))
            for ch in filler:
                ch()

    nc.compile()
    return nc


def host_inputs(x, v1, W_qkv, b_qkv, W_proj, b_proj, lamb, Bv=B, Tv=T):
    """Shard + preprocess full inputs into per-core input maps."""
    NT = Bv * Tv
    TPB = Tv // 128
    lam = float(lamb)

    xT = np.ascontiguousarray(np.asarray(x, np.float32).reshape(NT, C).T).astype(ml_dtypes.bfloat16)

    # rope tables, token-major per 128-tile: cs[p, u*32+f] with position u*128+p
    pos = (np.arange(TPB)[:, None, None] * 128 + np.arange(128)[None, :, None]).astype(np.float32)
    inv_freq = (1.0 / ROPE_BASE ** (np.arange(0, D, 2, dtype=np.float32) / D))[None, None, :]
    ang = pos * inv_freq                      # [TPB, 128, 32]
    cos_t = np.cos(ang).transpose(1, 0, 2).reshape(128, TPB * 32)
    sin_t = np.sin(ang).transpose(1, 0, 2).reshape(128, TPB * 32)
    cs = np.concatenate([cos_t, cos_t, sin_t, sin_t], axis=1).astype(ml_dtypes.bfloat16)
    cs = np.ascontiguousarray(cs)

    masku = np.triu(np.ones((128, 128), np.float32)).astype(ml_dtypes.bfloat16)
    ones = np.ones((1, 128), ml_dtypes.bfloat16)

    W_qkv = np.asarray(W_qkv, np.float32)
    b_qkv = np.asarray(b_qkv, np.float32)
    W_proj = np.asarray(W_proj, np.float32)
    v1 = np.asarray(v1, np.float32)

    in_maps = []
    for c in range(NCORES):
        r0 = c * HD
        Wq = W_qkv[r0:r0 + HD]
        Wk = W_qkv[C + r0:C + r0 + HD]
        Wv = W_qkv[2 * C + r0:2 * C + r0 + HD] * (1.0 - lam)
        wqkvT = np.ascontiguousarray(np.concatenate([Wq, Wk, Wv], axis=0).T).astype(ml_dtypes.bfloat16)
        brow = np.concatenate([
            b_qkv[r0:r0 + HD], b_qkv[C + r0:C + r0 + HD],
            b_qkv[2 * C + r0:2 * C + r0 + HD] * (1.0 - lam),
        ])[None, :].astype(np.float32)
        brow = np.ascontiguousarray(brow).astype(ml_dtypes.bfloat16)
        v1s = np.ascontiguousarray(
            (lam * v1[:, c * HPC:(c + 1) * HPC]).transpose(0, 2, 1, 3).reshape(NT, HD)
        )
        wp = np.ascontiguousarray(W_proj[:, r0:r0 + HD].T).astype(ml_dtypes.bfloat16)
        in_maps.append({
            "xT": xT, "wqkvT": wqkvT, "brow": brow, "ones": ones,
            "v1s": v1s, "cs": cs, "masku": masku, "wp": wp,
        })
    return in_maps


def host_gather(results, b_proj, Bv=B, Tv=T):
    NT = Bv * Tv
    out = np.zeros((NT, C), np.float32)
    for c in range(NCORES):
        out += results[c]["outp"]
    out += np.asarray(b_proj, np.float32)[None, :]
    out = out.reshape(Bv, Tv, C)
    value = np.empty((Bv, H, Tv, D), np.float32)
    for c in range(NCORES):
        value[:, c * HPC:(c + 1) * HPC] = (
            results[c]["val"].reshape(Bv, Tv, HPC, D).transpose(0, 2, 1, 3)
        )
    return out, value


_NC_CACHE = {}


def _get_module(Bv=B, Tv=T):
    key = (Bv, Tv)
    if key not in _NC_CACHE:
        _NC_CACHE[key] = build_module(Bv, Tv)
    return _NC_CACHE[key]


last_results = None


def kernel(x, v1, W_qkv, b_qkv, W_proj, b_proj, lamb, _trace=False):
    global last_results
    nc = _get_module()
    in_maps = host_inputs(x, v1, W_qkv, b_qkv, W_proj, b_proj, lamb)
    if _trace:
        _install_ntff_hook()
    res = run_bass_kernel_spmd(nc, in_maps, core_ids=list(range(NCORES)), trace=_trace)
    last_results = res
    return host_gather(res.results, b_proj)


def _install_ntff_hook():
    """Best-effort NTFF profiling hook for axon (used only when _trace=True)."""
    try:
        import types, sys
        if "antenv.axon_hooks" not in sys.modules:
            mod = types.ModuleType("antenv.axon_hooks")
            _h = [None]
            mod.set_axon_ntff_profile_hook = lambda h: _h.__setitem__(0, h)
            mod.get_axon_ntff_profile_hook = lambda: _h[0]
            sys.modules["antenv.axon_hooks"] = mod
        from antenv.axon_hooks import get_axon_ntff_profile_hook, set_axon_ntff_profile_hook
        if get_axon_ntff_profile_hook() is None:
            from trn_agent_boot.trn_boot import _ntff_profile_via_ctypes
            set_axon_ntff_profile_hook(_ntff_profile_via_ctypes("/opt/axon/libaxon_pjrt.so"))
    except Exception:
        pass


# revision 18
# speedup vs baseline: 1.0766x; 1.0354x over previous
"""Trainium2 Bass kernel for CausalSelfAttention (B=4, T=2048, C=1024, H=16, D=64).

Sharding: tensor-parallel over attention heads — 2 heads per core, 8 cores,
zero collectives. Each core computes QKV for its 2 heads (full token range),
runs causal attention, and produces a partial output projection
(its heads' columns of W_proj); the host sums the 8 partials and adds b_proj.
The mixed value tensor (an output of the module) is emitted per-core and
reassembled on the host.

Per-core dataflow (token-major QKV -> norm/rope -> PE transpose to dim-major
-> scores^T [k,q] -> exp (no max subtraction needed: qk-norm bounds scores)
-> attn@v with a ones-column to accumulate the softmax denominator ->
normalize -> output projection).
"""

import numpy as np
import ml_dtypes

import concourse.bass as bass
from concourse import bacc, mybir, tile, masks
from concourse.bass_utils import run_bass_kernel_spmd

dt = mybir.dt
AF = mybir.ActivationFunctionType
ALU = mybir.AluOpType

B, T, C, H, D = 4, 2048, 1024, 16, 64
NCORES = 8
HPC = H // NCORES          # heads per core
HD = HPC * D               # 128 head dims per core
ROPE_BASE = 10000.0
KC = C // 128              # contraction chunks for qkv proj


def build_module(Bv=B, Tv=T, debug_taps=False):
    """Build + compile the per-core Bass module. Identical on all cores (SPMD);
    only the input data differs per core."""
    NT = Bv * Tv
    TPB = Tv // 128        # token tiles per batch
    JPB = Tv // 512        # 512-wide q groups per batch
    QKW = 3 * HD           # 384 qkv output dims per core

    nc = bacc.Bacc("TRN2", target_bir_lowering=False, debug=False)
    if debug_taps:
        dbg_qT = nc.dram_tensor("dbg_qT", (128, NT), dt.bfloat16, kind="ExternalOutput").ap()
        dbg_kT = nc.dram_tensor("dbg_kT", (128, NT), dt.bfloat16, kind="ExternalOutput").ap()
        dbg_aT = nc.dram_tensor("dbg_aT", (128, NT), dt.bfloat16, kind="ExternalOutput").ap()
        dbg_es = nc.dram_tensor("dbg_es", (128, 1024), dt.bfloat16, kind="ExternalOutput").ap()
        dbg_psO = nc.dram_tensor("dbg_psO", (65, 512), dt.float32, kind="ExternalOutput").ap()

    xT_h = nc.dram_tensor("xT", (C, NT), dt.bfloat16, kind="ExternalInput").ap()
    wq_h = nc.dram_tensor("wqkvT", (C, QKW), dt.bfloat16, kind="ExternalInput").ap()
    br_h = nc.dram_tensor("brow", (1, QKW), dt.bfloat16, kind="ExternalInput").ap()
    on_h = nc.dram_tensor("ones", (1, 128), dt.bfloat16, kind="ExternalInput").ap()
    v1_h = nc.dram_tensor("v1s", (NT, HD), dt.float32, kind="ExternalInput").ap()
    cs_h = nc.dram_tensor("cs", (128, 4 * TPB * 32), dt.bfloat16, kind="ExternalInput").ap()
    mk_h = nc.dram_tensor("masku", (128, 128), dt.bfloat16, kind="ExternalInput").ap()
    wp_h = nc.dram_tensor("wp", (HD, C), dt.bfloat16, kind="ExternalInput").ap()
    outp_h = nc.dram_tensor("outp", (NT, C), dt.float32, kind="ExternalOutput").ap()
    val_h = nc.dram_tensor("val", (NT, HD), dt.float32, kind="ExternalOutput").ap()

    f32r = dt.float32r

    with tile.TileContext(nc) as tc:
        with (
            tc.tile_pool(name="const", bufs=1) as const,
            tc.tile_pool(name="work", bufs=1) as work,
            tc.tile_pool(name="io", bufs=1) as io,
            tc.tile_pool(name="ps", bufs=1, space="PSUM") as ps,
        ):
            # ---- constants / persistent tensors ----
            wq_sb = const.tile([128, KC * QKW], dt.bfloat16, name="wq_sb")
            for kc in range(KC):
                nc.sync.dma_start(
                    wq_sb[:, kc * QKW:(kc + 1) * QKW],
                    wq_h[kc * 128:(kc + 1) * 128, :],
                )
            br_sb = const.tile([1, QKW], dt.bfloat16, name="br_sb")
            nc.sync.dma_start(br_sb[:], br_h[:])
            on_sb = const.tile([1, 128], dt.bfloat16, name="on_sb")
            nc.sync.dma_start(on_sb[:], on_h[:])
            wp_sb = const.tile([HD, C], dt.bfloat16, name="wp_sb")
            nc.sync.dma_start(wp_sb[:], wp_h[:])
            cs_sb = const.tile([128, 4 * TPB * 32], dt.bfloat16, name="cs_sb")
            nc.sync.dma_start(cs_sb[:], cs_h[:])
            mk_sb = const.tile([128, 128], dt.bfloat16, name="mk_sb")
            nc.sync.dma_start(mk_sb[:], mk_h[:])
            id_sb = const.tile([128, 128], dt.bfloat16, name="id_sb")
            masks.make_identity(nc, id_sb[:])
            eps_sb = const.tile([128, 1], dt.float32, name="eps_sb")
            nc.vector.memset(eps_sb[:], 1e-6)

            qT_sb = const.tile([128, NT], dt.bfloat16, name="qT_sb")
            kT_sb = const.tile([128, NT], dt.bfloat16, name="kT_sb")
            aT_sb = const.tile([128, NT], dt.bfloat16, name="aT_sb")

            U = 2 * TPB  # rope groups (q tiles then k tiles)

            G = 2 * Tv // 64
            state = {}

            def phaseA_chunks(b):
                """QKV projection (token-major) for batch b, as per-m4 chunks."""
                qksb = work.tile([128, 2 * Tv], dt.float32, tag="qksb", bufs=2, name="qksb")
                vaug = work.tile([128, TPB * 130], dt.bfloat16, tag="vaug", bufs=2, name="vaug")
                nc.vector.memset(
                    vaug[:].rearrange("p (t h y) -> p t h y", h=2, y=65)[:, :, :, 64:65],
                    1.0,
                )
                state[("qksb", b)] = qksb
                state[("vaug", b)] = vaug

                def chunk(m4):
                    xt = io.tile([128, KC * 512], dt.bfloat16, tag="xt", bufs=3, name="xt")
                    g4 = b * TPB + m4 * 4
                    for kc in range(KC):
                        nc.sync.dma_start(
                            xt[:, kc * 512:(kc + 1) * 512],
                            xT_h[kc * 128:(kc + 1) * 128, g4 * 128:(g4 + 4) * 128],
                        )
                    v1t = io.tile([128, 512], dt.float32, tag="v1t", bufs=2, name="v1t")
                    nc.sync.dma_start(
                        v1t[:].rearrange("p (m d) -> p m d", d=HD),
                        v1_h[g4 * 128:(g4 + 4) * 128, :].rearrange("(m p) d -> p m d", p=128),
                    )
                    vmix4 = io.tile([128, 512], dt.float32, tag="vmix", bufs=2, name="vmix4")
                    for mi in range(4):
                        m = m4 * 4 + mi
                        ps_qkv = ps.tile([128, QKW], dt.float32, tag="mm", bufs=2, name="ps_qkv")
                        for kc in range(KC):
                            nc.tensor.matmul(
                                ps_qkv[:],
                                xt[:, kc * 512 + mi * 128: kc * 512 + (mi + 1) * 128],
                                wq_sb[:, kc * QKW:(kc + 1) * QKW],
                                start=(kc == 0),
                                stop=False,
                            )
                        nc.tensor.matmul(ps_qkv[:], on_sb[:], br_sb[:], start=False, stop=True)
                        evac_dst = qksb[:].rearrange("p (s t) -> p s t", s=2)[:, :, m * 128:(m + 1) * 128]
                        evac_src = ps_qkv[:, 0:256].rearrange("p (s u) -> p s u", s=2)
                        if m % 2 == 0:
                            nc.scalar.copy(evac_dst, evac_src)
                        else:
                            nc.vector.tensor_copy(evac_dst, evac_src)
                        nc.vector.tensor_tensor(
                            vmix4[:, mi * HD:(mi + 1) * HD], ps_qkv[:, 256:384],
                            v1t[:, mi * HD:(mi + 1) * HD], ALU.add,
                        )
                    nc.sync.dma_start(
                        val_h[g4 * 128:(g4 + 4) * 128, :].rearrange("(m p) d -> p m d", p=128),
                        vmix4[:].rearrange("p (m d) -> p m d", d=HD),
                    )
                    nc.vector.tensor_copy(
                        vaug[:, m4 * 520:(m4 + 1) * 520].rearrange("p (m h y) -> p m h y", m=4, y=65)[:, :, :, 0:64],
                        vmix4[:].rearrange("p (m h d) -> p m h d", h=2, d=64),
                    )

                return [(lambda mm: (lambda: chunk(mm)))(m4) for m4 in range(TPB // 4)]

            def phaseB(b):
                """rms-norm + rope for batch b."""
                qksb = state.pop(("qksb", b))
                sq = work.tile([128, 2 * Tv], dt.float32, tag="qkn", bufs=1, name="sq")
                nc.vector.tensor_tensor(sq[:], qksb[:], qksb[:], ALU.mult)
                ssum = work.tile([128, G], dt.float32, tag="ss", bufs=1, name="ssum")
                nc.vector.tensor_reduce(
                    ssum[:], sq[:].rearrange("p (g d) -> p g d", d=64),
                    mybir.AxisListType.X, ALU.add,
                )
                sfac = work.tile([128, G], dt.float32, tag="sf", bufs=1, name="sfac")
                nc.scalar.activation(sfac[:], ssum[:], AF.Sqrt, scale=1.0 / 64, bias=eps_sb[:])
                rfac = work.tile([128, G], dt.float32, tag="rf", bufs=1, name="rfac")
                nc.vector.reciprocal(rfac[:], sfac[:])
                qkn = work.tile([128, 2 * Tv], dt.bfloat16, tag="qkn2", bufs=1, name="qkn")
                nc.vector.tensor_tensor(
                    qkn[:].rearrange("p (g d) -> p g d", d=64),
                    qksb[:].rearrange("p (g d) -> p g d", d=64),
                    rfac[:].broadcast_to([128, G, 64]),
                    ALU.mult,
                )
                rbf = work.tile([128, 2 * Tv], dt.bfloat16, tag="rbf", bufs=1, name="rbf")
                qk4 = qkn[:].rearrange("p (u h d) -> p u h d", h=2, d=64)
                rb4 = rbf[:].rearrange("p (u h d) -> p u h d", h=2, d=64)
                cosv = cs_sb[:, 0:U * 32].rearrange("p (u f) -> p u f", f=32)
                sinv = cs_sb[:, 2 * TPB * 32:2 * TPB * 32 + U * 32].rearrange("p (u f) -> p u f", f=32)
                for h in range(2):
                    x1 = qk4[:, :, h, 0:32]
                    x2 = qk4[:, :, h, 32:64]
                    t1 = work.tile([128, U * 32], dt.bfloat16, tag="tmp", bufs=2, name="t1")
                    t2 = work.tile([128, U * 32], dt.bfloat16, tag="tmp", bufs=2, name="t2")
                    t1v = t1[:].rearrange("p (u f) -> p u f", f=32)
                    t2v = t2[:].rearrange("p (u f) -> p u f", f=32)
                    nc.vector.tensor_tensor(t1v, x1, cosv, ALU.mult)
                    nc.vector.tensor_tensor(t2v, x2, sinv, ALU.mult)
                    nc.vector.tensor_tensor(rb4[:, :, h, 0:32], t1v, t2v, ALU.add)
                    t3 = work.tile([128, U * 32], dt.bfloat16, tag="tmp", bufs=2, name="t3")
                    t4 = work.tile([128, U * 32], dt.bfloat16, tag="tmp", bufs=2, name="t4")
                    t3v = t3[:].rearrange("p (u f) -> p u f", f=32)
                    t4v = t4[:].rearrange("p (u f) -> p u f", f=32)
                    nc.vector.tensor_tensor(t3v, x2, cosv, ALU.mult)
                    nc.vector.tensor_tensor(t4v, x1, sinv, ALU.mult)
                    nc.vector.tensor_tensor(rb4[:, :, h, 32:64], t3v, t4v, ALU.subtract)
                state[("rbf", b)] = rbf

            def phaseC(b):
                """PE transpose to dim-major."""
                rbf = state.pop(("rbf", b))
                for half, dst in ((0, qT_sb), (1, kT_sb)):
                    for p4 in range(TPB // 4):  # noqa: loop kept flat
                        tp = ps.tile([128, 512], dt.bfloat16, tag="ao", bufs=2, name="tp")
                        for t4 in range(4):
                            m = p4 * 4 + t4
                            nc.tensor.transpose(
                                tp[:, t4 * 128:(t4 + 1) * 128],
                                rbf[:, half * Tv + m * 128: half * Tv + (m + 1) * 128],
                                id_sb[:],
                            )
                        nc.vector.tensor_copy(dst[:, b * Tv + p4 * 512: b * Tv + (p4 + 1) * 512], tp[:])

            def phaseD_chunks(b):
                """Causal attention for batch b, as per-(j, ip) chunks plus a
                normalize chunk per j."""
                vaug = state.pop(("vaug", b))
                chunks = []
                for j in range(JPB):
                    psO = [
                        ps.tile([65, 512], dt.float32, tag="ao", bufs=2, name=f"psO{h}")
                        for h in range(2)
                    ]
                    first_mm = [True, True]
                    qs = b * Tv + j * 512

                    def ip_chunk(j, ip, psO, first_mm, qs):
                        for h in range(2):
                            hs = slice(h * 64, (h + 1) * 64)
                            sc = ps.tile([128, 1024], dt.float32, tag="sc", bufs=2, name="sc")
                            for w in range(2):
                                i = ip + w
                                dd = i - 4 * j
                                kbase = b * Tv + i * 128
                                lo = max(dd, 0) * 128
                                nc.tensor.matmul(
                                    sc[:, w * 512 + lo: (w + 1) * 512],
                                    kT_sb[hs, kbase:kbase + 128],
                                    qT_sb[hs, qs + lo: qs + 512],
                                    start=True, stop=True,
                                )
                            eS = work.tile([128, 1024], dt.bfloat16, tag="es", bufs=4, name="eS")
                            dd0, dd1 = ip - 4 * j, ip + 1 - 4 * j
                            if dd1 <= 0:
                                nc.scalar.activation(eS[:], sc[:], AF.Exp, scale=0.125)
                            else:
                                lo0 = max(dd0, 0) * 128
                                nc.scalar.activation(eS[:, lo0:512], sc[:, lo0:512], AF.Exp, scale=0.125)
                                nc.scalar.activation(
                                    eS[:, 512 + dd1 * 128:1024], sc[:, 512 + dd1 * 128:1024],
                                    AF.Exp, scale=0.125,
                                )
                            for w in range(2):
                                dd = ip + w - 4 * j
                                if 0 <= dd <= 3:
                                    dcol = w * 512 + dd * 128
                                    nc.vector.tensor_tensor(
                                        eS[:, dcol:dcol + 128], eS[:, dcol:dcol + 128],
                                        mk_sb[:], ALU.mult,
                                    )
                            if debug_taps and b == 0 and j == 0 and ip == 0 and h == 0:
                                nc.sync.dma_start(dbg_es[:, 0:512], eS[:, 0:512])
                                nc.sync.dma_start(dbg_es[:, 640:1024], eS[:, 640:1024])
                            for w in range(2):
                                i = ip + w
                                dd = i - 4 * j
                                vsl = vaug[:, i * 130 + h * 65: i * 130 + (h + 1) * 65]
                                lo = max(dd, 0) * 128
                                nc.tensor.matmul(
                                    psO[h][:, lo:512], vsl, eS[:, w * 512 + lo: (w + 1) * 512],
                                    start=first_mm[h], stop=(dd == 3),
                                )
                                first_mm[h] = False
                    def norm_chunk(j, psO, qs):
                        if debug_taps and b == 0 and j == 0:
                            ps_dbg = io.tile([65, 512], dt.float32, tag="psdbg", bufs=1)
                            nc.vector.tensor_copy(ps_dbg[:], psO[0][:])
                            nc.sync.dma_start(dbg_psO[:], ps_dbg[:])
                        for h in range(2):
                            drow = work.tile([1, 512], dt.float32, tag="drow", bufs=2, name="drow")
                            nc.vector.tensor_copy(drow[:], psO[h][64:65, :])
                            rrow = work.tile([1, 512], dt.float32, tag="rrow", bufs=2, name="rrow")
                            nc.vector.reciprocal_approx_fast(rrow[:], drow[:])
                            rrep = work.tile([64, 512], dt.float32, tag="rrep", bufs=2, name="rrep")
                            nc.gpsimd.partition_broadcast(rrep[:], rrow[:])
                            nc.vector.tensor_tensor(
                                aT_sb[h * 64:(h + 1) * 64, qs:qs + 512],
                                psO[h][0:64, :], rrep[:], ALU.mult,
                            )

                    for ip in range(0, 4 * j + 4, 2):
                        chunks.append((lambda jj, pp, oo, ff, qq: (lambda: ip_chunk(jj, pp, oo, ff, qq)))(j, ip, psO, first_mm, qs))
                    chunks.append((lambda jj, oo, qq: (lambda: norm_chunk(jj, oo, qq)))(j, psO, qs))
                return chunks

            def phaseE_chunks(b):
                """Output projection (partial) for batch b, per-m4 chunks."""
                if debug_taps:
                    nc.sync.dma_start(dbg_qT[:, b * Tv:(b + 1) * Tv], qT_sb[:, b * Tv:(b + 1) * Tv])
                    nc.sync.dma_start(dbg_kT[:, b * Tv:(b + 1) * Tv], kT_sb[:, b * Tv:(b + 1) * Tv])
                    nc.sync.dma_start(dbg_aT[:, b * Tv:(b + 1) * Tv], aT_sb[:, b * Tv:(b + 1) * Tv])

                def e_chunk(m):
                    g = b * TPB + m
                    psP = ps.tile([128, 1024], dt.float32, tag="sc", bufs=2, name="psP")
                    for nn in range(C // 512):
                        nc.tensor.matmul(
                            psP[:, nn * 512:(nn + 1) * 512],
                            aT_sb[:, g * 128:(g + 1) * 128],
                            wp_sb[:, nn * 512:(nn + 1) * 512],
                            start=True, stop=True,
                        )
                    osb = io.tile([128, C], dt.float32, tag="osb", bufs=2, name="osb")
                    nc.scalar.copy(osb[:, 0:512], psP[:, 0:512])
                    nc.vector.tensor_copy(osb[:, 512:1024], psP[:, 512:1024])
                    nc.gpsimd.dma_start(outp_h[g * 128:(g + 1) * 128, :], osb[:])

                return [(lambda mm: (lambda: e_chunk(mm)))(m) for m in range(TPB)]

            # Software-pipelined, interleaved emission. During batch b's
            # attention (exp-latency bound on ACT), inject next batch's QKV
            # matmul chunks and previous batch's out-proj chunks into the PE
            # stream so the tensor engine stays dense (keeps HAM at K=8/8).
            for chunk in phaseA_chunks(0):
                chunk()
            filler = []
            for b in range(Bv):
                phaseB(b)
                # next batch's projection fills the PE while norm/rope runs on DVE
                if b + 1 < Bv:
                    for ch in phaseA_chunks(b + 1):
                        ch()
                phaseC(b)
                d_chunks = phaseD_chunks(b)
                n_d = len(d_chunks)
                n_f = len(filler)
                fi = 0
                for ci, ch in enumerate(d_chunks):
                    ch()
                    want = (ci + 1) * n_f // n_d
                    while fi < want:
                        filler[fi]()
                        fi += 1
                filler = list(phaseE_chunks(b))
            for ch in filler:
                ch()

    nc.compile()
    return nc


def host_inputs(x, v1, W_qkv, b_qkv, W_proj, b_proj, lamb, Bv=B, Tv=T):
    """Shard + preprocess full inputs into per-core input maps."""
    NT = Bv * Tv
    TPB = Tv // 128
    lam = float(lamb)

    xT = np.ascontiguousarray(np.asarray(x, np.float32).reshape(NT, C).T).astype(ml_dtypes.bfloat16)

    # rope tables, token-major per 128-tile: cs[p, u*32+f] with position u*128+p
    pos = (np.arange(TPB)[:, None, None] * 128 + np.arange(128)[None, :, None]).astype(np.float32)
    inv_freq = (1.0 / ROPE_BASE ** (np.arange(0, D, 2, dtype=np.float32) / D))[None, None, :]
    ang = pos * inv_freq                      # [TPB, 128, 32]
    cos_t = np.cos(ang).transpose(1, 0, 2).reshape(128, TPB * 32)
    sin_t = np.sin(ang).transpose(1, 0, 2).reshape(128, TPB * 32)
    cs = np.concatenate([cos_t, cos_t, sin_t, sin_t], axis=1).astype(ml_dtypes.bfloat16)
    cs = np.ascontiguousarray(cs)

    masku = np.triu(np.ones((128, 128), np.float32)).astype(ml_dtypes.bfloat16)
    ones = np.ones((1, 128), ml_dtypes.bfloat16)

    W_qkv = np.asarray(W_qkv, np.float32)
    b_qkv = np.asarray(b_qkv, np.float32)
    W_proj = np.asarray(W_proj, np.float32)
    v1 = np.asarray(v1, np.float32)

    in_maps = []
    for c in range(NCORES):
        r0 = c * HD
        Wq = W_qkv[r0:r0 + HD]
        Wk = W_qkv[C + r0:C + r0 + HD]
        Wv = W_qkv[2 * C + r0:2 * C + r0 + HD] * (1.0 - lam)
        wqkvT = np.ascontiguousarray(np.concatenate([Wq, Wk, Wv], axis=0).T).astype(ml_dtypes.bfloat16)
        brow = np.concatenate([
            b_qkv[r0:r0 + HD], b_qkv[C + r0:C + r0 + HD],
            b_qkv[2 * C + r0:2 * C + r0 + HD] * (1.0 - lam),
        ])[None, :].astype(np.float32)
        brow = np.ascontiguousarray(brow).astype(ml_dtypes.bfloat16)
        v1s = np.ascontiguousarray(
            (lam * v1[:, c * HPC:(c + 1) * HPC]).transpose(0, 2, 1, 3).reshape(NT, HD)
        )
        wp = np.ascontiguousarray(W_proj[:, r0:r0 + HD].T).astype(ml_dtypes.bfloat16)
        in_maps.append({
            "xT": xT, "wqkvT": wqkvT, "brow": brow, "ones": ones,
            "v1s": v1s, "cs": cs, "masku": masku, "wp": wp,
        })
    return in_maps


def host_gather(results, b_proj, Bv=B, Tv=T):
    NT = Bv * Tv
    out = np.zeros((NT, C), np.float32)
    for c in range(NCORES):
        out += results[c]["outp"]
    out += np.asarray(b_proj, np.float32)[None, :]
    out = out.reshape(Bv, Tv, C)
    value = np.empty((Bv, H, Tv, D), np.float32)
    for c in range(NCORES):
        value[:, c * HPC:(c + 1) * HPC] = (
            results[c]["val"].reshape(Bv, Tv, HPC, D).transpose(0, 2, 1, 3)
        )
    return out, value


_NC_CACHE = {}


def _get_module(Bv=B, Tv=T):
    key = (Bv, Tv)
    if key not in _NC_CACHE:
        _NC_CACHE[key] = build_module(Bv, Tv)
    return _NC_CACHE[key]


last_results = None


def kernel(x, v1, W_qkv, b_qkv, W_proj, b_proj, lamb, _trace=False):
    global last_results
    nc = _get_module()
    in_maps = host_inputs(x, v1, W_qkv, b_qkv, W_proj, b_proj, lamb)
    if _trace:
        _install_ntff_hook()
    res = run_bass_kernel_spmd(nc, in_maps, core_ids=list(range(NCORES)), trace=_trace)
    last_results = res
    return host_gather(res.results, b_proj)


def _install_ntff_hook():
    """Best-effort NTFF profiling hook for axon (used only when _trace=True)."""
    try:
        import types, sys
        if "antenv.axon_hooks" not in sys.modules:
            mod = types.ModuleType("antenv.axon_hooks")
            _h = [None]
            mod.set_axon_ntff_profile_hook = lambda h: _h.__setitem__(0, h)
            mod.get_axon_ntff_profile_hook = lambda: _h[0]
            sys.modules["antenv.axon_hooks"] = mod
        from antenv.axon_hooks import get_axon_ntff_profile_hook, set_axon_ntff_profile_hook
        if get_axon_ntff_profile_hook() is None:
            from trn_agent_boot.trn_boot import _ntff_profile_via_ctypes
            set_axon_ntff_profile_hook(_ntff_profile_via_ctypes("/opt/axon/libaxon_pjrt.so"))
    except Exception:
        pass


# revision 19
# speedup vs baseline: 1.1276x; 1.0474x over previous
"""Trainium2 Bass kernel for CausalSelfAttention (B=4, T=2048, C=1024, H=16, D=64).

Sharding: tensor-parallel over attention heads — 2 heads per core, 8 cores,
zero collectives. Each core computes QKV for its 2 heads (full token range),
runs causal attention, and produces a partial output projection
(its heads' columns of W_proj); the host sums the 8 partials and adds b_proj.
The mixed value tensor (an output of the module) is emitted per-core and
reassembled on the host.

Per-core dataflow (token-major QKV -> norm/rope -> PE transpose to dim-major
-> scores^T [k,q] -> exp (no max subtraction needed: qk-norm bounds scores)
-> attn@v with a ones-column to accumulate the softmax denominator ->
normalize -> output projection).
"""

import numpy as np
import ml_dtypes

import concourse.bass as bass
from concourse import bacc, mybir, tile, masks
from concourse.bass_utils import run_bass_kernel_spmd

dt = mybir.dt
AF = mybir.ActivationFunctionType
ALU = mybir.AluOpType

B, T, C, H, D = 4, 2048, 1024, 16, 64
NCORES = 8
HPC = H // NCORES          # heads per core
HD = HPC * D               # 128 head dims per core
ROPE_BASE = 10000.0
KC = C // 128              # contraction chunks for qkv proj


def build_module(Bv=B, Tv=T, debug_taps=False):
    """Build + compile the per-core Bass module. Identical on all cores (SPMD);
    only the input data differs per core."""
    NT = Bv * Tv
    TPB = Tv // 128        # token tiles per batch
    JPB = Tv // 512        # 512-wide q groups per batch
    QKW = 3 * HD           # 384 qkv output dims per core

    nc = bacc.Bacc("TRN2", target_bir_lowering=False, debug=False)
    if debug_taps:
        dbg_qT = nc.dram_tensor("dbg_qT", (128, NT), dt.bfloat16, kind="ExternalOutput").ap()
        dbg_kT = nc.dram_tensor("dbg_kT", (128, NT), dt.bfloat16, kind="ExternalOutput").ap()
        dbg_aT = nc.dram_tensor("dbg_aT", (128, NT), dt.bfloat16, kind="ExternalOutput").ap()
        dbg_es = nc.dram_tensor("dbg_es", (128, 1024), dt.bfloat16, kind="ExternalOutput").ap()
        dbg_psO = nc.dram_tensor("dbg_psO", (65, 512), dt.float32, kind="ExternalOutput").ap()

    xT_h = nc.dram_tensor("xT", (C, NT), dt.bfloat16, kind="ExternalInput").ap()
    wq_h = nc.dram_tensor("wqkvT", (C, QKW), dt.bfloat16, kind="ExternalInput").ap()
    br_h = nc.dram_tensor("brow", (1, QKW), dt.bfloat16, kind="ExternalInput").ap()
    on_h = nc.dram_tensor("ones", (1, 128), dt.bfloat16, kind="ExternalInput").ap()
    v1_h = nc.dram_tensor("v1s", (NT, HD), dt.float32, kind="ExternalInput").ap()
    cs_h = nc.dram_tensor("cs", (128, 4 * TPB * 32), dt.bfloat16, kind="ExternalInput").ap()
    mk_h = nc.dram_tensor("masku", (128, 128), dt.bfloat16, kind="ExternalInput").ap()
    wp_h = nc.dram_tensor("wp", (HD, C), dt.bfloat16, kind="ExternalInput").ap()
    outp_h = nc.dram_tensor("outp", (NT, C), dt.float32, kind="ExternalOutput").ap()
    val_h = nc.dram_tensor("val", (NT, HD), dt.float32, kind="ExternalOutput").ap()

    f32r = dt.float32r

    with tile.TileContext(nc) as tc:
        with (
            tc.tile_pool(name="const", bufs=1) as const,
            tc.tile_pool(name="work", bufs=1) as work,
            tc.tile_pool(name="io", bufs=1) as io,
            tc.tile_pool(name="ps", bufs=1, space="PSUM") as ps,
        ):
            # ---- constants / persistent tensors ----
            wq_sb = const.tile([128, KC * QKW], dt.bfloat16, name="wq_sb")
            for kc in range(KC):
                nc.sync.dma_start(
                    wq_sb[:, kc * QKW:(kc + 1) * QKW],
                    wq_h[kc * 128:(kc + 1) * 128, :],
                )
            br_sb = const.tile([1, QKW], dt.bfloat16, name="br_sb")
            nc.sync.dma_start(br_sb[:], br_h[:])
            on_sb = const.tile([1, 128], dt.bfloat16, name="on_sb")
            nc.sync.dma_start(on_sb[:], on_h[:])
            wp_sb = const.tile([HD, C], dt.bfloat16, name="wp_sb")
            nc.sync.dma_start(wp_sb[:], wp_h[:])
            cs_sb = const.tile([128, 4 * TPB * 32], dt.bfloat16, name="cs_sb")
            nc.sync.dma_start(cs_sb[:], cs_h[:])
            mk_sb = const.tile([128, 128], dt.bfloat16, name="mk_sb")
            nc.sync.dma_start(mk_sb[:], mk_h[:])
            id_sb = const.tile([128, 128], dt.bfloat16, name="id_sb")
            masks.make_identity(nc, id_sb[:])
            eps_sb = const.tile([128, 1], dt.float32, name="eps_sb")
            nc.vector.memset(eps_sb[:], 1e-6)

            qT_sb = const.tile([128, NT], dt.bfloat16, name="qT_sb")
            kT_sb = const.tile([128, NT], dt.bfloat16, name="kT_sb")
            aT_sb = const.tile([128, NT], dt.bfloat16, name="aT_sb")

            U = 2 * TPB  # rope groups (q tiles then k tiles)

            G = 2 * Tv // 64
            state = {}

            def phaseA_chunks(b):
                """QKV projection (token-major) for batch b, as per-m4 chunks."""
                qksb = work.tile([128, 2 * Tv], dt.float32, tag="qksb", bufs=2, name="qksb")
                vaug = work.tile([128, TPB * 130], dt.bfloat16, tag="vaug", bufs=2, name="vaug")
                nc.vector.memset(
                    vaug[:].rearrange("p (t h y) -> p t h y", h=2, y=65)[:, :, :, 64:65],
                    1.0,
                )
                state[("qksb", b)] = qksb
                state[("vaug", b)] = vaug

                def chunk(m4):
                    xt = io.tile([128, KC * 512], dt.bfloat16, tag="xt", bufs=3, name="xt")
                    g4 = b * TPB + m4 * 4
                    for kc in range(KC):
                        nc.sync.dma_start(
                            xt[:, kc * 512:(kc + 1) * 512],
                            xT_h[kc * 128:(kc + 1) * 128, g4 * 128:(g4 + 4) * 128],
                        )
                    v1t = io.tile([128, 512], dt.float32, tag="v1t", bufs=2, name="v1t")
                    nc.sync.dma_start(
                        v1t[:].rearrange("p (m d) -> p m d", d=HD),
                        v1_h[g4 * 128:(g4 + 4) * 128, :].rearrange("(m p) d -> p m d", p=128),
                    )
                    vmix4 = io.tile([128, 512], dt.float32, tag="vmix", bufs=2, name="vmix4")
                    for mi in range(4):
                        m = m4 * 4 + mi
                        ps_qkv = ps.tile([128, QKW], dt.float32, tag="mm", bufs=2, name="ps_qkv")
                        for kc in range(KC):
                            nc.tensor.matmul(
                                ps_qkv[:],
                                xt[:, kc * 512 + mi * 128: kc * 512 + (mi + 1) * 128],
                                wq_sb[:, kc * QKW:(kc + 1) * QKW],
                                start=(kc == 0),
                                stop=False,
                            )
                        nc.tensor.matmul(ps_qkv[:], on_sb[:], br_sb[:], start=False, stop=True)
                        evac_dst = qksb[:].rearrange("p (s t) -> p s t", s=2)[:, :, m * 128:(m + 1) * 128]
                        evac_src = ps_qkv[:, 0:256].rearrange("p (s u) -> p s u", s=2)
                        if m % 2 == 0:
                            nc.scalar.copy(evac_dst, evac_src)
                        else:
                            nc.vector.tensor_copy(evac_dst, evac_src)
                        nc.vector.tensor_tensor(
                            vmix4[:, mi * HD:(mi + 1) * HD], ps_qkv[:, 256:384],
                            v1t[:, mi * HD:(mi + 1) * HD], ALU.add,
                        )
                    nc.sync.dma_start(
                        val_h[g4 * 128:(g4 + 4) * 128, :].rearrange("(m p) d -> p m d", p=128),
                        vmix4[:].rearrange("p (m d) -> p m d", d=HD),
                    )
                    nc.vector.tensor_copy(
                        vaug[:, m4 * 520:(m4 + 1) * 520].rearrange("p (m h y) -> p m h y", m=4, y=65)[:, :, :, 0:64],
                        vmix4[:].rearrange("p (m h d) -> p m h d", h=2, d=64),
                    )

                return [(lambda mm: (lambda: chunk(mm)))(m4) for m4 in range(TPB // 4)]

            def phaseB(b):
                """rms-norm + rope for batch b."""
                qksb = state.pop(("qksb", b))
                sq = work.tile([128, 2 * Tv], dt.float32, tag="qkn", bufs=1, name="sq")
                nc.vector.tensor_tensor(sq[:], qksb[:], qksb[:], ALU.mult)
                ssum = work.tile([128, G], dt.float32, tag="ss", bufs=1, name="ssum")
                nc.vector.tensor_reduce(
                    ssum[:], sq[:].rearrange("p (g d) -> p g d", d=64),
                    mybir.AxisListType.X, ALU.add,
                )
                sfac = work.tile([128, G], dt.float32, tag="sf", bufs=1, name="sfac")
                nc.scalar.activation(sfac[:], ssum[:], AF.Sqrt, scale=1.0 / 64, bias=eps_sb[:])
                rfac = work.tile([128, G], dt.float32, tag="rf", bufs=1, name="rfac")
                nc.vector.reciprocal(rfac[:], sfac[:])
                qkn = work.tile([128, 2 * Tv], dt.bfloat16, tag="qkn2", bufs=1, name="qkn")
                nc.vector.tensor_tensor(
                    qkn[:].rearrange("p (g d) -> p g d", d=64),
                    qksb[:].rearrange("p (g d) -> p g d", d=64),
                    rfac[:].broadcast_to([128, G, 64]),
                    ALU.mult,
                )
                rbf = work.tile([128, 2 * Tv], dt.bfloat16, tag="rbf", bufs=1, name="rbf")
                qk4 = qkn[:].rearrange("p (u h d) -> p u h d", h=2, d=64)
                rb4 = rbf[:].rearrange("p (u h d) -> p u h d", h=2, d=64)
                cosv = cs_sb[:, 0:U * 32].rearrange("p (u f) -> p u f", f=32)
                sinv = cs_sb[:, 2 * TPB * 32:2 * TPB * 32 + U * 32].rearrange("p (u f) -> p u f", f=32)
                for h in range(2):
                    x1 = qk4[:, :, h, 0:32]
                    x2 = qk4[:, :, h, 32:64]
                    t1 = work.tile([128, U * 32], dt.bfloat16, tag="tmp", bufs=2, name="t1")
                    t2 = work.tile([128, U * 32], dt.bfloat16, tag="tmp", bufs=2, name="t2")
                    t1v = t1[:].rearrange("p (u f) -> p u f", f=32)
                    t2v = t2[:].rearrange("p (u f) -> p u f", f=32)
                    nc.vector.tensor_tensor(t1v, x1, cosv, ALU.mult)
                    nc.vector.tensor_tensor(t2v, x2, sinv, ALU.mult)
                    nc.vector.tensor_tensor(rb4[:, :, h, 0:32], t1v, t2v, ALU.add)
                    t3 = work.tile([128, U * 32], dt.bfloat16, tag="tmp", bufs=2, name="t3")
                    t4 = work.tile([128, U * 32], dt.bfloat16, tag="tmp", bufs=2, name="t4")
                    t3v = t3[:].rearrange("p (u f) -> p u f", f=32)
                    t4v = t4[:].rearrange("p (u f) -> p u f", f=32)
                    nc.vector.tensor_tensor(t3v, x2, cosv, ALU.mult)
                    nc.vector.tensor_tensor(t4v, x1, sinv, ALU.mult)
                    nc.vector.tensor_tensor(rb4[:, :, h, 32:64], t3v, t4v, ALU.subtract)
                state[("rbf", b)] = rbf

            def phaseC(b):
                """PE transpose to dim-major."""
                rbf = state.pop(("rbf", b))
                for half, dst in ((0, qT_sb), (1, kT_sb)):
                    for p4 in range(TPB // 4):  # noqa: loop kept flat
                        tp = ps.tile([128, 512], dt.bfloat16, tag="ao", bufs=2, name="tp")
                        for t4 in range(4):
                            m = p4 * 4 + t4
                            nc.tensor.transpose(
                                tp[:, t4 * 128:(t4 + 1) * 128],
                                rbf[:, half * Tv + m * 128: half * Tv + (m + 1) * 128],
                                id_sb[:],
                            )
                        nc.vector.tensor_copy(dst[:, b * Tv + p4 * 512: b * Tv + (p4 + 1) * 512], tp[:])

            def phaseD_chunks(b):
                """Causal attention for batch b, as per-(j, ip) chunks plus a
                normalize chunk per j."""
                vaug = state.pop(("vaug", b))
                chunks = []
                for j in range(JPB):
                    psO = [
                        ps.tile([65, 512], dt.float32, tag="ao", bufs=2, name=f"psO{h}")
                        for h in range(2)
                    ]
                    first_mm = [True, True]
                    qs = b * Tv + j * 512

                    def se_chunk(j, ip, qs, esd):
                        for h in range(2):
                            hs = slice(h * 64, (h + 1) * 64)
                            sc = ps.tile([128, 1024], dt.float32, tag="sc", bufs=2, name="sc")
                            for w in range(2):
                                i = ip + w
                                dd = i - 4 * j
                                kbase = b * Tv + i * 128
                                lo = max(dd, 0) * 128
                                nc.tensor.matmul(
                                    sc[:, w * 512 + lo: (w + 1) * 512],
                                    kT_sb[hs, kbase:kbase + 128],
                                    qT_sb[hs, qs + lo: qs + 512],
                                    start=True, stop=True,
                                )
                            eS = work.tile([128, 1024], dt.bfloat16, tag="es", bufs=5, name="eS")
                            dd0, dd1 = ip - 4 * j, ip + 1 - 4 * j
                            if dd1 <= 0:
                                nc.scalar.activation(eS[:], sc[:], AF.Exp, scale=0.125)
                            else:
                                lo0 = max(dd0, 0) * 128
                                nc.scalar.activation(eS[:, lo0:512], sc[:, lo0:512], AF.Exp, scale=0.125)
                                nc.scalar.activation(
                                    eS[:, 512 + dd1 * 128:1024], sc[:, 512 + dd1 * 128:1024],
                                    AF.Exp, scale=0.125,
                                )
                            for w in range(2):
                                dd = ip + w - 4 * j
                                if 0 <= dd <= 3:
                                    dcol = w * 512 + dd * 128
                                    nc.vector.tensor_tensor(
                                        eS[:, dcol:dcol + 128], eS[:, dcol:dcol + 128],
                                        mk_sb[:], ALU.mult,
                                    )
                            if debug_taps and b == 0 and j == 0 and ip == 0 and h == 0:
                                nc.sync.dma_start(dbg_es[:, 0:512], eS[:, 0:512])
                                nc.sync.dma_start(dbg_es[:, 640:1024], eS[:, 640:1024])
                            esd[(ip, h)] = eS

                    def av_chunk(j, ip, psO, first_mm, qs, esd):
                        for h in range(2):
                            eS = esd.pop((ip, h))
                            for w in range(2):
                                i = ip + w
                                dd = i - 4 * j
                                vsl = vaug[:, i * 130 + h * 65: i * 130 + (h + 1) * 65]
                                lo = max(dd, 0) * 128
                                nc.tensor.matmul(
                                    psO[h][:, lo:512], vsl, eS[:, w * 512 + lo: (w + 1) * 512],
                                    start=first_mm[h], stop=(dd == 3),
                                )
                                first_mm[h] = False
                    def norm_chunk(j, psO, qs):
                        if debug_taps and b == 0 and j == 0:
                            ps_dbg = io.tile([65, 512], dt.float32, tag="psdbg", bufs=1)
                            nc.vector.tensor_copy(ps_dbg[:], psO[0][:])
                            nc.sync.dma_start(dbg_psO[:], ps_dbg[:])
                        for h in range(2):
                            drow = work.tile([1, 512], dt.float32, tag="drow", bufs=2, name="drow")
                            nc.vector.tensor_copy(drow[:], psO[h][64:65, :])
                            rrow = work.tile([1, 512], dt.float32, tag="rrow", bufs=2, name="rrow")
                            nc.vector.reciprocal_approx_fast(rrow[:], drow[:])
                            rrep = work.tile([64, 512], dt.float32, tag="rrep", bufs=2, name="rrep")
                            nc.gpsimd.partition_broadcast(rrep[:], rrow[:])
                            nc.vector.tensor_tensor(
                                aT_sb[h * 64:(h + 1) * 64, qs:qs + 512],
                                psO[h][0:64, :], rrep[:], ALU.mult,
                            )

                    esd = {}
                    ips = list(range(0, 4 * j + 4, 2))
                    mk_se = lambda jj, pp, qq: (lambda: se_chunk(jj, pp, qq, esd))
                    mk_av = lambda jj, pp, oo, ff, qq: (lambda: av_chunk(jj, pp, oo, ff, qq, esd))
                    chunks.append(mk_se(j, ips[0], qs))
                    for idx in range(1, len(ips)):
                        chunks.append(mk_se(j, ips[idx], qs))
                        chunks.append(mk_av(j, ips[idx - 1], psO, first_mm, qs))
                    chunks.append(mk_av(j, ips[-1], psO, first_mm, qs))
                    chunks.append((lambda jj, oo, qq: (lambda: norm_chunk(jj, oo, qq)))(j, psO, qs))
                return chunks

            def phaseE_chunks(b):
                """Output projection (partial) for batch b, per-m4 chunks."""
                if debug_taps:
                    nc.sync.dma_start(dbg_qT[:, b * Tv:(b + 1) * Tv], qT_sb[:, b * Tv:(b + 1) * Tv])
                    nc.sync.dma_start(dbg_kT[:, b * Tv:(b + 1) * Tv], kT_sb[:, b * Tv:(b + 1) * Tv])
                    nc.sync.dma_start(dbg_aT[:, b * Tv:(b + 1) * Tv], aT_sb[:, b * Tv:(b + 1) * Tv])

                def e_chunk(m):
                    g = b * TPB + m
                    psP = ps.tile([128, 1024], dt.float32, tag="sc", bufs=2, name="psP")
                    for nn in range(C // 512):
                        nc.tensor.matmul(
                            psP[:, nn * 512:(nn + 1) * 512],
                            aT_sb[:, g * 128:(g + 1) * 128],
                            wp_sb[:, nn * 512:(nn + 1) * 512],
                            start=True, stop=True,
                        )
                    osb = io.tile([128, C], dt.float32, tag="osb", bufs=2, name="osb")
                    nc.vector.tensor_copy(osb[:], psP[:])
                    nc.gpsimd.dma_start(outp_h[g * 128:(g + 1) * 128, :], osb[:])

                return [(lambda mm: (lambda: e_chunk(mm)))(m) for m in range(TPB)]

            # Software-pipelined, interleaved emission. During batch b's
            # attention (exp-latency bound on ACT), inject next batch's QKV
            # matmul chunks and previous batch's out-proj chunks into the PE
            # stream so the tensor engine stays dense (keeps HAM at K=8/8).
            for chunk in phaseA_chunks(0):
                chunk()
            filler = []
            for b in range(Bv):
                phaseB(b)
                # next batch's projection fills the PE while norm/rope runs on DVE
                if b + 1 < Bv:
                    for ch in phaseA_chunks(b + 1):
                        ch()
                phaseC(b)
                d_chunks = phaseD_chunks(b)
                n_d = len(d_chunks)
                n_f = len(filler)
                fi = 0
                for ci, ch in enumerate(d_chunks):
                    ch()
                    want = (ci + 1) * n_f // n_d
                    while fi < want:
                        filler[fi]()
                        fi += 1
                filler = list(phaseE_chunks(b))
            for ch in filler:
                ch()

    nc.compile()
    return nc


def host_inputs(x, v1, W_qkv, b_qkv, W_proj, b_proj, lamb, Bv=B, Tv=T):
    """Shard + preprocess full inputs into per-core input maps."""
    NT = Bv * Tv
    TPB = Tv // 128
    lam = float(lamb)

    xT = np.ascontiguousarray(np.asarray(x, np.float32).reshape(NT, C).T).astype(ml_dtypes.bfloat16)

    # rope tables, token-major per 128-tile: cs[p, u*32+f] with position u*128+p
    pos = (np.arange(TPB)[:, None, None] * 128 + np.arange(128)[None, :, None]).astype(np.float32)
    inv_freq = (1.0 / ROPE_BASE ** (np.arange(0, D, 2, dtype=np.float32) / D))[None, None, :]
    ang = pos * inv_freq                      # [TPB, 128, 32]
    cos_t = np.cos(ang).transpose(1, 0, 2).reshape(128, TPB * 32)
    sin_t = np.sin(ang).transpose(1, 0, 2).reshape(128, TPB * 32)
    cs = np.concatenate([cos_t, cos_t, sin_t, sin_t], axis=1).astype(ml_dtypes.bfloat16)
    cs = np.ascontiguousarray(cs)

    masku = np.triu(np.ones((128, 128), np.float32)).astype(ml_dtypes.bfloat16)
    ones = np.ones((1, 128), ml_dtypes.bfloat16)

    W_qkv = np.asarray(W_qkv, np.float32)
    b_qkv = np.asarray(b_qkv, np.float32)
    W_proj = np.asarray(W_proj, np.float32)
    v1 = np.asarray(v1, np.float32)

    in_maps = []
    for c in range(NCORES):
        r0 = c * HD
        Wq = W_qkv[r0:r0 + HD]
        Wk = W_qkv[C + r0:C + r0 + HD]
        Wv = W_qkv[2 * C + r0:2 * C + r0 + HD] * (1.0 - lam)
        wqkvT = np.ascontiguousarray(np.concatenate([Wq, Wk, Wv], axis=0).T).astype(ml_dtypes.bfloat16)
        brow = np.concatenate([
            b_qkv[r0:r0 + HD], b_qkv[C + r0:C + r0 + HD],
            b_qkv[2 * C + r0:2 * C + r0 + HD] * (1.0 - lam),
        ])[None, :].astype(np.float32)
        brow = np.ascontiguousarray(brow).astype(ml_dtypes.bfloat16)
        v1s = np.ascontiguousarray(
            (lam * v1[:, c * HPC:(c + 1) * HPC]).transpose(0, 2, 1, 3).reshape(NT, HD)
        )
        wp = np.ascontiguousarray(W_proj[:, r0:r0 + HD].T).astype(ml_dtypes.bfloat16)
        in_maps.append({
            "xT": xT, "wqkvT": wqkvT, "brow": brow, "ones": ones,
            "v1s": v1s, "cs": cs, "masku": masku, "wp": wp,
        })
    return in_maps


def host_gather(results, b_proj, Bv=B, Tv=T):
    NT = Bv * Tv
    out = np.zeros((NT, C), np.float32)
    for c in range(NCORES):
        out += results[c]["outp"]
    out += np.asarray(b_proj, np.float32)[None, :]
    out = out.reshape(Bv, Tv, C)
    value = np.empty((Bv, H, Tv, D), np.float32)
    for c in range(NCORES):
        value[:, c * HPC:(c + 1) * HPC] = (
            results[c]["val"].reshape(Bv, Tv, HPC, D).transpose(0, 2, 1, 3)
        )
    return out, value


_NC_CACHE = {}


def _get_module(Bv=B, Tv=T):
    key = (Bv, Tv)
    if key not in _NC_CACHE:
        _NC_CACHE[key] = build_module(Bv, Tv)
    return _NC_CACHE[key]


last_results = None


def kernel(x, v1, W_qkv, b_qkv, W_proj, b_proj, lamb, _trace=False):
    global last_results
    nc = _get_module()
    in_maps = host_inputs(x, v1, W_qkv, b_qkv, W_proj, b_proj, lamb)
    if _trace:
        _install_ntff_hook()
    res = run_bass_kernel_spmd(nc, in_maps, core_ids=list(range(NCORES)), trace=_trace)
    last_results = res
    return host_gather(res.results, b_proj)


def _install_ntff_hook():
    """Best-effort NTFF profiling hook for axon (used only when _trace=True)."""
    try:
        import types, sys
        if "antenv.axon_hooks" not in sys.modules:
            mod = types.ModuleType("antenv.axon_hooks")
            _h = [None]
            mod.set_axon_ntff_profile_hook = lambda h: _h.__setitem__(0, h)
            mod.get_axon_ntff_profile_hook = lambda: _h[0]
            sys.modules["antenv.axon_hooks"] = mod
        from antenv.axon_hooks import get_axon_ntff_profile_hook, set_axon_ntff_profile_hook
        if get_axon_ntff_profile_hook() is None:
            from trn_agent_boot.trn_boot import _ntff_profile_via_ctypes
            set_axon_ntff_profile_hook(_ntff_profile_via_ctypes("/opt/axon/libaxon_pjrt.so"))
    except Exception:
        pass
